# revision 3
# baseline (speedup 1.0000x reference)
"""Trainium2 Bass kernel for nn_AttentionLayer (additive/Bahdanau-style attention).

Reference computation:
  qp = query @ W1[:H] ; kp = key @ W1[H:]          # [B,S1,HM], [B,S2,HM]
  hid = relu(qp[:,:,None,:] + kp[:,None,:,:] + b1)  # [B,S1,S2,HM]
  scores = hid @ W2 + b2                            # [B,S1,S2]
  A = where(qmask*kmask==0, 0, exp(scores))
  out = (A / clip(A.sum(-1), 2e-15)) @ value        # [B,S1,H]

Sharding: data-parallel over batch, 2 batches per core on 8 cores.

Sparsity: masked q rows produce all-zero output rows and masked k columns
contribute exactly zero to every sum, so the host compacts both dimensions
per batch (keeping only mask==1 rows, zero-padded up to a common per-call
capacity) and scatters the result back. The device still applies the k-mask
(padding lanes carry mask 0), so results are exact.

Device mapping (per core, 2 batches):
  stage0 (PE):   qpT[h,q], kpT[h,k] projections (+b1), kpT cast bf16
  pair  (DVE/ACT): hidT[h,k] = relu(kpT_hb + qpT_hb[:,q]) per (b,q,hb), one
                 fused per-partition-scalar op; both batches share one wide
                 [128, 2*KKP] tile so the PE consumes them in one matmul
  scores (PE):   matmul with zero-padded-column weights Z(hb,j)[128,32]
                 (col j = W2 h-block) -> scores row lands on PSUM partition
                 q=32*cg+j (tile_position col groups), fp32 accumulation
  post:          Exp(+b2) over both batches at once; PE transpose -> A_T[k,q];
                 k_mask applied on PSUM evict; out_un = A_T.T @ [value | ones]
                 (ones column = row-sum); out = out_un * 1/clip(rowsum).
"""

import os
import sys

import numpy as np

for _p in ("/opt/trn_rl_repo",):
    if os.path.isdir(_p) and _p not in sys.path:
        sys.path.insert(0, _p)

import ml_dtypes  # noqa: E402
import concourse.bass as bass  # noqa: E402
import concourse.mybir as mybir  # noqa: E402
import concourse.tile as tile  # noqa: E402
from concourse import bacc  # noqa: E402
from concourse.bass_utils import run_bass_kernel_spmd  # noqa: E402
from concourse.masks import make_identity  # noqa: E402

B, S1, S2, H, HM = 16, 128, 256, 256, 512
N_CORES = 8
BPC = B // N_CORES  # batches per core
NHB = HM // 128  # h blocks
NDC = H // 128  # d chunks (projection contraction)
VA = H + 2  # value dims + ones column + pad
ACT_EVERY = 4  # every ACT_EVERY-th pair op runs on the scalar engine

FP32 = mybir.dt.float32
BF16 = mybir.dt.bfloat16
ADD = mybir.AluOpType.add
MAX = mybir.AluOpType.max
RELU = mybir.ActivationFunctionType.Relu
EXP = mybir.ActivationFunctionType.Exp

_cache: dict = {}


def _build(QNP, KKP):
    """Build + compile the per-core program for q-capacity QNP, k-capacity KKP."""
    key = (QNP, KKP)
    if key in _cache:
        return _cache[key]

    CGC = (QNP + 31) // 32  # col groups used
    QP32 = CGC * 32  # partition extent of the scores region
    KC = (KKP + 127) // 128  # k chunks for transpose/value
    W2K = 2 * KKP  # wide (both batches) k extent

    nc = bacc.Bacc("TRN2", target_bir_lowering=False, debug=False, num_devices=1)

    qT = nc.dram_tensor("qT", [BPC, H, QNP], FP32, kind="ExternalInput")
    kT = nc.dram_tensor("kT", [BPC, H, KKP], FP32, kind="ExternalInput")
    vaug = nc.dram_tensor("vaug", [BPC, KKP, VA], FP32, kind="ExternalInput")
    w1 = nc.dram_tensor("w1", [2 * H, HM], FP32, kind="ExternalInput")
    b1T = nc.dram_tensor("b1T", [NHB, 128, 1], FP32, kind="ExternalInput")
    zmat = nc.dram_tensor("zmat", [128, NHB * 1024], BF16, kind="ExternalInput")
    kmT = nc.dram_tensor("kmT", [BPC, KKP, 1], FP32, kind="ExternalInput")
    b2c = nc.dram_tensor("b2c", [128, 1], FP32, kind="ExternalInput")
    y = nc.dram_tensor("y", [BPC, QP32, H], FP32, kind="ExternalOutput")

    with tile.TileContext(nc) as tc:
        with (
            tc.tile_pool(name="const", bufs=1) as cp,
            tc.tile_pool(name="acts", bufs=1) as ap,
            tc.tile_pool(name="hid", bufs=12) as hp,
            tc.tile_pool(name="small", bufs=2) as sp,
            tc.tile_pool(name="psA", bufs=2, space=bass.MemorySpace.PSUM) as psA,
            tc.tile_pool(name="psS", bufs=1, space=bass.MemorySpace.PSUM) as psS,
            tc.tile_pool(name="psB", bufs=2, space=bass.MemorySpace.PSUM) as psB,
        ):
            # ---------------- constants / inputs to SBUF ----------------
            w1q, w1k = {}, {}
            for dc in range(NDC):
                for hb in range(NHB):
                    t = cp.tile([128, 128], FP32, name=f"w1q{dc}{hb}", tag=f"w1q{dc}{hb}")
                    nc.sync.dma_start(
                        t[:], w1.ap()[dc * 128 : (dc + 1) * 128, hb * 128 : (hb + 1) * 128]
                    )
                    w1q[dc, hb] = t
                    t = cp.tile([128, 128], FP32, name=f"w1k{dc}{hb}", tag=f"w1k{dc}{hb}")
                    nc.sync.dma_start(
                        t[:],
                        w1.ap()[H + dc * 128 : H + (dc + 1) * 128, hb * 128 : (hb + 1) * 128],
                    )
                    w1k[dc, hb] = t
            qT_sb, kT_sb, va_sb = {}, {}, {}
            for b in range(BPC):
                for dc in range(NDC):
                    t = cp.tile([128, QNP], FP32, name=f"qT{b}{dc}", tag=f"qT{b}{dc}")
                    nc.sync.dma_start(t[:], qT.ap()[b, dc * 128 : (dc + 1) * 128, :])
                    qT_sb[b, dc] = t
                    t = cp.tile([128, KKP], FP32, name=f"kT{b}{dc}", tag=f"kT{b}{dc}")
                    nc.sync.dma_start(t[:], kT.ap()[b, dc * 128 : (dc + 1) * 128, :])
                    kT_sb[b, dc] = t
                for kc in range(KC):
                    kw = min(128, KKP - kc * 128)
                    t = cp.tile([128, VA], FP32, name=f"va{b}{kc}", tag=f"va{b}{kc}")
                    nc.sync.dma_start(t[:kw, :], vaug.ap()[b, kc * 128 : kc * 128 + kw, :])
                    va_sb[b, kc] = t
            zm = cp.tile([128, NHB * 1024], BF16, name="zm", tag="zm")
            nc.sync.dma_start(zm[:], zmat.ap())
            b1_sb = {}
            for hb in range(NHB):
                t = cp.tile([128, 1], FP32, name=f"b1{hb}", tag=f"b1{hb}")
                nc.sync.dma_start(t[:], b1T.ap()[hb])
                b1_sb[hb] = t
            km_sb = {}
            for b in range(BPC):
                for kc in range(KC):
                    kw = min(128, KKP - kc * 128)
                    t = cp.tile([128, 1], FP32, name=f"km{b}{kc}", tag=f"km{b}{kc}")
                    nc.sync.dma_start(t[:kw, :], kmT.ap()[b, kc * 128 : kc * 128 + kw, :])
                    km_sb[b, kc] = t
            b2_sb = cp.tile([128, 1], FP32, name="b2", tag="b2")
            nc.sync.dma_start(b2_sb[:], b2c.ap())
            ident = cp.tile([128, 128], FP32, name="ident", tag="ident")
            make_identity(nc, ident[:])

            # ---------------- stage 0: projections ----------------
            qpT, kpB = {}, {}
            for b in range(BPC):
                for hb in range(NHB):
                    ps = psA.tile([128, KKP], FP32, name="proj", tag="proj")
                    for dc in range(NDC):
                        nc.tensor.matmul(
                            ps[:, :QNP],
                            w1q[dc, hb][:],
                            qT_sb[b, dc][:],
                            start=(dc == 0),
                            stop=(dc == NDC - 1),
                        )
                    t = ap.tile([128, QNP], FP32, name=f"qpT{b}{hb}", tag=f"qpT{b}{hb}")
                    nc.vector.tensor_scalar_add(t[:], ps[:, :QNP], b1_sb[hb][:, 0:1])
                    qpT[b, hb] = t
                    ps2 = psA.tile([128, KKP], FP32, name="proj", tag="proj")
                    for dc in range(NDC):
                        nc.tensor.matmul(
                            ps2[:],
                            w1k[dc, hb][:],
                            kT_sb[b, dc][:],
                            start=(dc == 0),
                            stop=(dc == NDC - 1),
                        )
                    t2 = ap.tile([128, KKP], BF16, name=f"kpB{b}{hb}", tag=f"kpB{b}{hb}")
                    nc.vector.tensor_scalar_add(t2[:], ps2[:], b1_sb[hb][:, 0:1])
                    kpB[b, hb] = t2

            # ---------------- pair stage + score reduce ----------------
            scores = psS.tile([128, W2K], FP32, name="scps", tag="scps")
            cnt = 0
            for hb in range(NHB):
                for j in range(32):
                    w_ap = zm[:, hb * 1024 + j * 32 : hb * 1024 + (j + 1) * 32]
                    for cg in range(CGC):
                        q = cg * 32 + j
                        if q >= QNP:
                            continue
                        hid = hp.tile([128, W2K], BF16, name="hid", tag="hid")
                        for b in range(BPC):
                            qcol = qpT[b, hb][:, q : q + 1]
                            dst = hid[:, b * KKP : (b + 1) * KKP]
                            if cnt % ACT_EVERY == ACT_EVERY - 1:
                                nc.scalar.activation(
                                    dst, kpB[b, hb][:], RELU, bias=qcol, scale=1.0
                                )
                            else:
                                nc.vector.tensor_scalar(
                                    dst, kpB[b, hb][:], qcol, 0.0, ADD, MAX
                                )
                            cnt += 1
                        jlast = min(31, QNP - 1 - cg * 32)
                        nc.tensor.matmul(
                            scores[cg * 32 : (cg + 1) * 32, :],
                            w_ap,
                            hid[:],
                            start=(hb == 0 and j == 0),
                            stop=(hb == NHB - 1 and j == jlast),
                            tile_position=(0, cg * 32),
                        )

            # ---------------- exp / transpose / mask / value matmul ----------------
            A = ap.tile([128, W2K], FP32, name="Aexp", tag="Aexp")
            nc.scalar.activation(
                A[:QP32, :], scores[:QP32, :], EXP, bias=b2_sb[:QP32, 0:1], scale=1.0
            )
            for b in range(BPC):
                AT = {}
                for kc in range(KC):
                    kw = min(128, KKP - kc * 128)
                    pst = psB.tile([128, QP32], FP32, name="trps", tag="trps")
                    nc.tensor.transpose(
                        pst[:kw, :],
                        A[:QP32, b * KKP + kc * 128 : b * KKP + kc * 128 + kw],
                        ident[:QP32, :QP32],
                    )
                    at = ap.tile([128, QP32], FP32, name=f"AT{b}{kc}", tag=f"AT{b}{kc}")
                    nc.vector.tensor_scalar_mul(at[:kw, :], pst[:kw, :], km_sb[b, kc][:kw, 0:1])
                    AT[kc] = at
                pso = psB.tile([128, VA], FP32, name="oun", tag="oun")
                for kc in range(KC):
                    kw = min(128, KKP - kc * 128)
                    nc.tensor.matmul(
                        pso[:QP32, :],
                        AT[kc][:kw, :],
                        va_sb[b, kc][:kw, :],
                        start=(kc == 0),
                        stop=(kc == KC - 1),
                    )
                asum = sp.tile([128, 1], FP32, name="asum", tag="asum")
                nc.vector.tensor_scalar_max(asum[:QP32, :], pso[:QP32, H : H + 1], 2e-15)
                rec = sp.tile([128, 1], FP32, name="rec", tag="rec")
                nc.vector.reciprocal(rec[:QP32, :], asum[:QP32, :])
                outt = ap.tile([128, H], FP32, name=f"out{b}", tag=f"out{b}")
                nc.vector.tensor_scalar_mul(outt[:QP32, :], pso[:QP32, 0:H], rec[:QP32, 0:1])
                nc.sync.dma_start(y.ap()[b], outt[:QP32, :])

    nc.compile()
    _cache[key] = nc
    return nc


def _round_up(x, m):
    return ((max(x, 1) + m - 1) // m) * m


def _prep(query, key, value, q_mask, k_mask, W1, b1, W2, b2):
    query = np.asarray(query, np.float32)
    key = np.asarray(key, np.float32)
    value = np.asarray(value, np.float32)
    q_mask = np.asarray(q_mask, np.float32)
    k_mask = np.asarray(k_mask, np.float32)
    W1 = np.ascontiguousarray(np.asarray(W1, np.float32))
    b1 = np.asarray(b1, np.float32)
    W2 = np.asarray(W2, np.float32)
    b2 = np.asarray(b2, np.float32)

    q_idx = [np.nonzero(q_mask[i] != 0)[0] for i in range(B)]
    k_idx = [np.nonzero(k_mask[i] != 0)[0] for i in range(B)]
    QNP = _round_up(max(len(ix) for ix in q_idx), 4)
    KKP = _round_up(max(len(ix) for ix in k_idx), 8)

    # compacted, padded host-side arrays
    qTc = np.zeros((B, H, QNP), np.float32)
    kTc = np.zeros((B, H, KKP), np.float32)
    vaug = np.zeros((B, KKP, VA), np.float32)
    kmc = np.zeros((B, KKP, 1), np.float32)
    for i in range(B):
        qi, ki = q_idx[i], k_idx[i]
        if len(qi):
            qTc[i, :, : len(qi)] = query[i, qi, :].T
        if len(ki):
            kTc[i, :, : len(ki)] = key[i, ki, :].T
            vaug[i, : len(ki), :H] = value[i, ki, :]
            vaug[i, : len(ki), H] = 1.0
            kmc[i, : len(ki), 0] = 1.0

    b1T = np.ascontiguousarray(b1.reshape(NHB, 128, 1))
    zmat = np.zeros((128, NHB * 1024), np.float32)
    for hb in range(NHB):
        for j in range(32):
            zmat[:, hb * 1024 + j * 32 + j] = W2[hb * 128 : (hb + 1) * 128, 0]
    zmat = zmat.astype(ml_dtypes.bfloat16)
    b2c = np.full((128, 1), float(b2[0]), np.float32)

    in_maps = []
    for c in range(N_CORES):
        sl = slice(BPC * c, BPC * (c + 1))
        in_maps.append(
            {
                "qT": np.ascontiguousarray(qTc[sl]),
                "kT": np.ascontiguousarray(kTc[sl]),
                "vaug": np.ascontiguousarray(vaug[sl]),
                "w1": W1,
                "b1T": b1T,
                "zmat": zmat,
                "kmT": np.ascontiguousarray(kmc[sl]),
                "b2c": b2c,
            }
        )
    return in_maps, q_idx, QNP, KKP


def kernel(query, key, value, q_mask, k_mask, W1, b1, W2, b2):
    in_maps, q_idx, QNP, KKP = _prep(
        query, key, value, q_mask, k_mask, W1, b1, W2, b2
    )
    nc = _build(QNP, KKP)
    res = run_bass_kernel_spmd(nc, in_maps, core_ids=list(range(N_CORES)))
    out = np.zeros((B, S1, H), np.float32)
    for c in range(N_CORES):
        yv = res.results[c]["y"]
        for b in range(BPC):
            gi = BPC * c + b
            qi = q_idx[gi]
            if len(qi):
                out[gi, qi, :] = yv[b, : len(qi), :]
    return out


def traced_single_core(query, key, value, q_mask, k_mask, W1, b1, W2, b2, core=0):
    """Run one core's share with NTFF tracing; returns (out, exec_time_ns)."""
    in_maps, q_idx, QNP, KKP = _prep(
        query, key, value, q_mask, k_mask, W1, b1, W2, b2
    )
    nc = _build(QNP, KKP)
    res = run_bass_kernel_spmd(nc, [in_maps[core]], core_ids=[0], trace=True)
    out = np.zeros((BPC, S1, H), np.float32)
    yv = res.results[0]["y"]
    for b in range(BPC):
        gi = BPC * core + b
        qi = q_idx[gi]
        if len(qi):
            out[b, qi, :] = yv[b, : len(qi), :]
    return out, res.exec_time_ns


# revision 4
# speedup vs baseline: 1.1415x; 1.1415x over previous
"""Trainium2 Bass kernel for nn_AttentionLayer (additive/Bahdanau-style attention).

Reference computation:
  qp = query @ W1[:H] ; kp = key @ W1[H:]          # [B,S1,HM], [B,S2,HM]
  hid = relu(qp[:,:,None,:] + kp[:,None,:,:] + b1)  # [B,S1,S2,HM]
  scores = hid @ W2 + b2                            # [B,S1,S2]
  A = where(qmask*kmask==0, 0, exp(scores))
  out = (A / clip(A.sum(-1), 2e-15)) @ value        # [B,S1,H]

Sharding: data-parallel over batch, 2 batches per core on 8 cores.

Sparsity: masked q rows produce all-zero output rows and masked k columns
contribute exactly zero to every sum, so the host compacts both dimensions
per batch (keeping only mask==1 rows, zero-padded up to a common per-call
capacity) and scatters the result back. The device still applies the k-mask
(padding lanes carry mask 0), so results are exact.

Device mapping (per core, 2 batches):
  stage0 (PE):   qpT[h,q], kpT[h,k] projections (+b1), kpT cast bf16
  pair  (DVE/ACT): hidT[h,k] = relu(kpT_hb + qpT_hb[:,q]) per (b,q,hb), one
                 fused per-partition-scalar op; both batches share one wide
                 [128, 2*KKP] tile so the PE consumes them in one matmul
  scores (PE):   matmul with zero-padded-column weights Z(hb,j)[128,32]
                 (col j = W2 h-block) -> scores row lands on PSUM partition
                 q=32*cg+j (tile_position col groups), fp32 accumulation
  post:          Exp(+b2) over both batches at once; PE transpose -> A_T[k,q];
                 k_mask applied on PSUM evict; out_un = A_T.T @ [value | ones]
                 (ones column = row-sum); out = out_un * 1/clip(rowsum).
"""

import os
import sys

import numpy as np

for _p in ("/opt/trn_rl_repo",):
    if os.path.isdir(_p) and _p not in sys.path:
        sys.path.insert(0, _p)

import ml_dtypes  # noqa: E402
import concourse.bass as bass  # noqa: E402
import concourse.mybir as mybir  # noqa: E402
import concourse.tile as tile  # noqa: E402
from concourse import bacc  # noqa: E402
from concourse.bass_utils import run_bass_kernel_spmd  # noqa: E402
from concourse.masks import make_identity  # noqa: E402

B, S1, S2, H, HM = 16, 128, 256, 256, 512
N_CORES = 8
BPC = B // N_CORES  # batches per core
NHB = HM // 128  # h blocks
NDC = H // 128  # d chunks (projection contraction)
VA = H + 2  # value dims + ones column + pad
ACT_PAT = (0, 4, 8, 12)  # which cnt%15 residues run on the scalar engine

FP32 = mybir.dt.float32
BF16 = mybir.dt.bfloat16
ADD = mybir.AluOpType.add
MAX = mybir.AluOpType.max
RELU = mybir.ActivationFunctionType.Relu
EXP = mybir.ActivationFunctionType.Exp

_cache: dict = {}


def _build(QNP, KKP):
    """Build + compile the per-core program for q-capacity QNP, k-capacity KKP."""
    key = (QNP, KKP)
    if key in _cache:
        return _cache[key]

    CGC = (QNP + 31) // 32  # col groups used
    QP32 = CGC * 32  # partition extent of the scores region
    KC = (KKP + 127) // 128  # k chunks for transpose/value
    W2K = 2 * KKP  # wide (both batches) k extent

    nc = bacc.Bacc("TRN2", target_bir_lowering=False, debug=False, num_devices=1)

    qT = nc.dram_tensor("qT", [BPC, H, QNP], BF16, kind="ExternalInput")
    kT = nc.dram_tensor("kT", [BPC, H, KKP], BF16, kind="ExternalInput")
    vaug = nc.dram_tensor("vaug", [BPC, KKP, VA], FP32, kind="ExternalInput")
    w1 = nc.dram_tensor("w1", [2 * H, HM], BF16, kind="ExternalInput")
    b1T = nc.dram_tensor("b1T", [NHB, 128, 1], FP32, kind="ExternalInput")
    zmr = nc.dram_tensor("zmr", [128, NHB * 64], BF16, kind="ExternalInput")
    kmT = nc.dram_tensor("kmT", [BPC, KKP, 1], FP32, kind="ExternalInput")
    b2c = nc.dram_tensor("b2c", [128, 1], FP32, kind="ExternalInput")
    y = nc.dram_tensor("y", [BPC, QP32, H], FP32, kind="ExternalOutput")

    with tile.TileContext(nc) as tc:
        with (
            tc.tile_pool(name="const", bufs=1) as cp,
            tc.tile_pool(name="acts", bufs=1) as ap,
            tc.tile_pool(name="hid", bufs=20) as hp,
            tc.tile_pool(name="small", bufs=2) as sp,
            tc.tile_pool(name="psA", bufs=2, space=bass.MemorySpace.PSUM) as psA,
            tc.tile_pool(name="psS", bufs=1, space=bass.MemorySpace.PSUM) as psS,
            tc.tile_pool(name="psB", bufs=2, space=bass.MemorySpace.PSUM) as psB,
        ):
            # ---------------- constants / inputs to SBUF ----------------
            w1q, w1k = {}, {}
            for dc in range(NDC):
                tq = cp.tile([128, HM], BF16, name=f"w1qs{dc}", tag=f"w1qs{dc}")
                nc.sync.dma_start(tq[:], w1.ap()[dc * 128 : (dc + 1) * 128, :])
                tk = cp.tile([128, HM], BF16, name=f"w1ks{dc}", tag=f"w1ks{dc}")
                nc.sync.dma_start(tk[:], w1.ap()[H + dc * 128 : H + (dc + 1) * 128, :])
                for hb in range(NHB):
                    w1q[dc, hb] = tq[:, hb * 128 : (hb + 1) * 128]
                    w1k[dc, hb] = tk[:, hb * 128 : (hb + 1) * 128]
            qT_sb, kT_sb, va_sb = {}, {}, {}
            for b in range(BPC):
                for dc in range(NDC):
                    t = cp.tile([128, QNP], BF16, name=f"qT{b}{dc}", tag=f"qT{b}{dc}")
                    nc.sync.dma_start(t[:], qT.ap()[b, dc * 128 : (dc + 1) * 128, :])
                    qT_sb[b, dc] = t
                    t = cp.tile([128, KKP], BF16, name=f"kT{b}{dc}", tag=f"kT{b}{dc}")
                    nc.sync.dma_start(t[:], kT.ap()[b, dc * 128 : (dc + 1) * 128, :])
                    kT_sb[b, dc] = t
                for kc in range(KC):
                    kw = min(128, KKP - kc * 128)
                    t = cp.tile([128, VA], FP32, name=f"va{b}{kc}", tag=f"va{b}{kc}")
                    nc.sync.dma_start(t[:kw, :], vaug.ap()[b, kc * 128 : kc * 128 + kw, :])
                    va_sb[b, kc] = t
            zm = cp.tile([128, NHB * 64], BF16, name="zm", tag="zm")
            nc.sync.dma_start(zm[:], zmr.ap())
            b1_sb = {}
            for hb in range(NHB):
                t = cp.tile([128, 1], FP32, name=f"b1{hb}", tag=f"b1{hb}")
                nc.sync.dma_start(t[:], b1T.ap()[hb])
                b1_sb[hb] = t
            km_sb = {}
            for b in range(BPC):
                for kc in range(KC):
                    kw = min(128, KKP - kc * 128)
                    t = cp.tile([128, 1], FP32, name=f"km{b}{kc}", tag=f"km{b}{kc}")
                    nc.sync.dma_start(t[:kw, :], kmT.ap()[b, kc * 128 : kc * 128 + kw, :])
                    km_sb[b, kc] = t
            b2_sb = cp.tile([128, 1], FP32, name="b2", tag="b2")
            nc.sync.dma_start(b2_sb[:], b2c.ap())
            ident = cp.tile([128, 128], FP32, name="ident", tag="ident")
            make_identity(nc, ident[:])

            # ---------------- stage 0: projections ----------------
            IDENT_F = mybir.ActivationFunctionType.Identity
            qpT, kpB = {}, {}
            for hb in range(NHB):
                for b in range(BPC):
                    ps = psA.tile([128, KKP], FP32, name="proj", tag="proj")
                    for dc in range(NDC):
                        nc.tensor.matmul(
                            ps[:, :QNP],
                            w1q[dc, hb],
                            qT_sb[b, dc][:],
                            start=(dc == 0),
                            stop=(dc == NDC - 1),
                        )
                    t = ap.tile([128, QNP], FP32, name=f"qpT{b}{hb}", tag=f"qpT{b}{hb}")
                    nc.scalar.activation(t[:], ps[:, :QNP], IDENT_F, bias=b1_sb[hb][:, 0:1])
                    qpT[b, hb] = t
                    ps2 = psA.tile([128, KKP], FP32, name="proj", tag="proj")
                    for dc in range(NDC):
                        nc.tensor.matmul(
                            ps2[:],
                            w1k[dc, hb],
                            kT_sb[b, dc][:],
                            start=(dc == 0),
                            stop=(dc == NDC - 1),
                        )
                    t2 = ap.tile([128, KKP], BF16, name=f"kpB{b}{hb}", tag=f"kpB{b}{hb}")
                    nc.scalar.activation(t2[:], ps2[:], IDENT_F, bias=b1_sb[hb][:, 0:1])
                    kpB[b, hb] = t2

            # ---------------- pair stage + score reduce ----------------
            scores = psS.tile([128, W2K], FP32, name="scps", tag="scps")
            cnt = 0
            for hb in range(NHB):
                for j in range(32):
                    w_ap = zm[:, hb * 64 + (31 - j) : hb * 64 + (63 - j)]
                    for cg in range(CGC):
                        q = cg * 32 + j
                        if q >= QNP:
                            continue
                        hid = hp.tile([128, W2K], BF16, name="hid", tag="hid")
                        for b in range(BPC):
                            qcol = qpT[b, hb][:, q : q + 1]
                            dst = hid[:, b * KKP : (b + 1) * KKP]
                            if cnt % 15 in ACT_PAT:
                                nc.scalar.activation(
                                    dst, kpB[b, hb][:], RELU, bias=qcol, scale=1.0
                                )
                            else:
                                nc.vector.tensor_scalar(
                                    dst, kpB[b, hb][:], qcol, 0.0, ADD, MAX
                                )
                            cnt += 1
                        jlast = min(31, QNP - 1 - cg * 32)
                        nc.tensor.matmul(
                            scores[cg * 32 : (cg + 1) * 32, :],
                            w_ap,
                            hid[:],
                            start=(hb == 0 and j == 0),
                            stop=(hb == NHB - 1 and j == jlast),
                            tile_position=(0, cg * 32),
                        )

            # ---------------- exp / transpose / mask / value matmul ----------------
            A = ap.tile([128, W2K], FP32, name="Aexp", tag="Aexp")
            nc.scalar.activation(
                A[:QP32, :], scores[:QP32, :], EXP, bias=b2_sb[:QP32, 0:1], scale=1.0
            )
            for b in range(BPC):
                AT = {}
                for kc in range(KC):
                    kw = min(128, KKP - kc * 128)
                    pst = psB.tile([128, QP32], FP32, name="trps", tag="trps")
                    nc.tensor.transpose(
                        pst[:kw, :],
                        A[:QP32, b * KKP + kc * 128 : b * KKP + kc * 128 + kw],
                        ident[:QP32, :QP32],
                    )
                    at = ap.tile([128, QP32], FP32, name=f"AT{b}{kc}", tag=f"AT{b}{kc}")
                    nc.scalar.activation(at[:kw, :], pst[:kw, :], IDENT_F, scale=km_sb[b, kc][:kw, 0:1])
                    AT[kc] = at
                pso = psB.tile([128, VA], FP32, name="oun", tag="oun")
                for kc in range(KC):
                    kw = min(128, KKP - kc * 128)
                    nc.tensor.matmul(
                        pso[:QP32, :],
                        AT[kc][:kw, :],
                        va_sb[b, kc][:kw, :],
                        start=(kc == 0),
                        stop=(kc == KC - 1),
                    )
                asum = sp.tile([128, 1], FP32, name="asum", tag="asum")
                nc.vector.tensor_scalar_max(asum[:QP32, :], pso[:QP32, H : H + 1], 2e-15)
                rec = sp.tile([128, 1], FP32, name="rec", tag="rec")
                nc.vector.reciprocal(rec[:QP32, :], asum[:QP32, :])
                outt = ap.tile([128, H], FP32, name=f"out{b}", tag=f"out{b}")
                nc.scalar.activation(outt[:QP32, :], pso[:QP32, 0:H], IDENT_F, scale=rec[:QP32, 0:1])
                nc.sync.dma_start(y.ap()[b], outt[:QP32, :])

    nc.compile()
    _cache[key] = nc
    return nc


def _round_up(x, m):
    return ((max(x, 1) + m - 1) // m) * m


def _prep(query, key, value, q_mask, k_mask, W1, b1, W2, b2):
    query = np.asarray(query, np.float32)
    key = np.asarray(key, np.float32)
    value = np.asarray(value, np.float32)
    q_mask = np.asarray(q_mask, np.float32)
    k_mask = np.asarray(k_mask, np.float32)
    W1 = np.ascontiguousarray(np.asarray(W1, ml_dtypes.bfloat16))
    b1 = np.asarray(b1, np.float32)
    W2 = np.asarray(W2, np.float32)
    b2 = np.asarray(b2, np.float32)

    q_idx = [np.nonzero(q_mask[i] != 0)[0] for i in range(B)]
    k_idx = [np.nonzero(k_mask[i] != 0)[0] for i in range(B)]
    QNP = _round_up(max(len(ix) for ix in q_idx), 4)
    KKP = _round_up(max(len(ix) for ix in k_idx), 8)

    # compacted, padded host-side arrays
    qTc = np.zeros((B, H, QNP), ml_dtypes.bfloat16)
    kTc = np.zeros((B, H, KKP), ml_dtypes.bfloat16)
    vaug = np.zeros((B, KKP, VA), np.float32)
    kmc = np.zeros((B, KKP, 1), np.float32)
    for i in range(B):
        qi, ki = q_idx[i], k_idx[i]
        if len(qi):
            qTc[i, :, : len(qi)] = query[i, qi, :].T.astype(ml_dtypes.bfloat16)
        if len(ki):
            kTc[i, :, : len(ki)] = key[i, ki, :].T.astype(ml_dtypes.bfloat16)
            vaug[i, : len(ki), :H] = value[i, ki, :]
            vaug[i, : len(ki), H] = 1.0
            kmc[i, : len(ki), 0] = 1.0

    b1T = np.ascontiguousarray(b1.reshape(NHB, 128, 1))
    zmr = np.zeros((128, NHB * 64), np.float32)
    for hb in range(NHB):
        zmr[:, hb * 64 + 31] = W2[hb * 128 : (hb + 1) * 128, 0]
    zmr = zmr.astype(ml_dtypes.bfloat16)
    b2c = np.full((128, 1), float(b2[0]), np.float32)

    in_maps = []
    for c in range(N_CORES):
        sl = slice(BPC * c, BPC * (c + 1))
        in_maps.append(
            {
                "qT": np.ascontiguousarray(qTc[sl]),
                "kT": np.ascontiguousarray(kTc[sl]),
                "vaug": np.ascontiguousarray(vaug[sl]),
                "w1": W1,
                "b1T": b1T,
                "zmr": zmr,
                "kmT": np.ascontiguousarray(kmc[sl]),
                "b2c": b2c,
            }
        )
    return in_maps, q_idx, QNP, KKP


def kernel(query, key, value, q_mask, k_mask, W1, b1, W2, b2):
    in_maps, q_idx, QNP, KKP = _prep(
        query, key, value, q_mask, k_mask, W1, b1, W2, b2
    )
    nc = _build(QNP, KKP)
    res = run_bass_kernel_spmd(nc, in_maps, core_ids=list(range(N_CORES)))
    out = np.zeros((B, S1, H), np.float32)
    for c in range(N_CORES):
        yv = res.results[c]["y"]
        for b in range(BPC):
            gi = BPC * c + b
            qi = q_idx[gi]
            if len(qi):
                out[gi, qi, :] = yv[b, : len(qi), :]
    return out


def traced_single_core(query, key, value, q_mask, k_mask, W1, b1, W2, b2, core=0):
    """Run one core's share with NTFF tracing; returns (out, exec_time_ns)."""
    in_maps, q_idx, QNP, KKP = _prep(
        query, key, value, q_mask, k_mask, W1, b1, W2, b2
    )
    nc = _build(QNP, KKP)
    res = run_bass_kernel_spmd(nc, [in_maps[core]], core_ids=[0], trace=True)
    out = np.zeros((BPC, S1, H), np.float32)
    yv = res.results[0]["y"]
    for b in range(BPC):
        gi = BPC * core + b
        qi = q_idx[gi]
        if len(qi):
            out[b, qi, :] = yv[b, : len(qi), :]
    return out, res.exec_time_ns


# revision 5
# speedup vs baseline: 1.1740x; 1.0284x over previous
"""Trainium2 Bass kernel for nn_AttentionLayer (additive/Bahdanau-style attention).

Reference computation:
  qp = query @ W1[:H] ; kp = key @ W1[H:]          # [B,S1,HM], [B,S2,HM]
  hid = relu(qp[:,:,None,:] + kp[:,None,:,:] + b1)  # [B,S1,S2,HM]
  scores = hid @ W2 + b2                            # [B,S1,S2]
  A = where(qmask*kmask==0, 0, exp(scores))
  out = (A / clip(A.sum(-1), 2e-15)) @ value        # [B,S1,H]

Sharding: data-parallel over batch, 2 batches per core on 8 cores.

Sparsity: masked q rows produce all-zero output rows and masked k columns
contribute exactly zero to every sum, so the host compacts both dimensions
per batch (keeping only mask==1 rows, zero-padded up to a common per-call
capacity) and scatters the result back. The device still applies the k-mask
(padding lanes carry mask 0), so results are exact.

Device mapping (per core, 2 batches):
  stage0 (PE):   qpT[h,q], kpT[h,k] projections (+b1), kpT cast bf16
  pair  (DVE/ACT): hidT[h,k] = relu(kpT_hb + qpT_hb[:,q]) per (b,q,hb), one
                 fused per-partition-scalar op; both batches share one wide
                 [128, 2*KKP] tile so the PE consumes them in one matmul
  scores (PE):   matmul with zero-padded-column weights Z(hb,j)[128,32]
                 (col j = W2 h-block) -> scores row lands on PSUM partition
                 q=32*cg+j (tile_position col groups), fp32 accumulation
  post:          Exp(+b2) over both batches at once; PE transpose -> A_T[k,q];
                 k_mask applied on PSUM evict; out_un = A_T.T @ [value | ones]
                 (ones column = row-sum); out = out_un * 1/clip(rowsum).
"""

import os
import sys

import numpy as np

for _p in ("/opt/trn_rl_repo",):
    if os.path.isdir(_p) and _p not in sys.path:
        sys.path.insert(0, _p)

import ml_dtypes  # noqa: E402
import concourse.bass as bass  # noqa: E402
import concourse.mybir as mybir  # noqa: E402
import concourse.tile as tile  # noqa: E402
from concourse import bacc  # noqa: E402
from concourse.bass_utils import run_bass_kernel_spmd  # noqa: E402
from concourse.masks import make_identity  # noqa: E402

B, S1, S2, H, HM = 16, 128, 256, 256, 512
N_CORES = 8
BPC = B // N_CORES  # batches per core
NHB = HM // 128  # h blocks
NDC = H // 128  # d chunks (projection contraction)
VA = H + 2  # value dims + ones column + pad
ACT_PAT = (0, 4, 8, 12)  # which cnt%15 residues run on the scalar engine

FP32 = mybir.dt.float32
BF16 = mybir.dt.bfloat16
ADD = mybir.AluOpType.add
MAX = mybir.AluOpType.max
RELU = mybir.ActivationFunctionType.Relu
EXP = mybir.ActivationFunctionType.Exp

_cache: dict = {}


def _build(QNP, KKP):
    """Build + compile the per-core program for q-capacity QNP, k-capacity KKP."""
    key = (QNP, KKP)
    if key in _cache:
        return _cache[key]

    CGC = (QNP + 31) // 32  # col groups used
    QP32 = CGC * 32  # partition extent of the scores region
    KC = (KKP + 127) // 128  # k chunks for transpose/value
    W2K = 2 * KKP  # wide (both batches) k extent

    nc = bacc.Bacc("TRN2", target_bir_lowering=False, debug=False, num_devices=1)

    qT = nc.dram_tensor("qT", [BPC, H, QNP], BF16, kind="ExternalInput")
    kT = nc.dram_tensor("kT", [BPC, H, KKP], BF16, kind="ExternalInput")
    vaug = nc.dram_tensor("vaug", [BPC, KKP, VA], FP32, kind="ExternalInput")
    w1 = nc.dram_tensor("w1", [2 * H, HM], BF16, kind="ExternalInput")
    b1T = nc.dram_tensor("b1T", [NHB, 128, 1], FP32, kind="ExternalInput")
    zmr = nc.dram_tensor("zmr", [128, NHB * 64], BF16, kind="ExternalInput")
    kmT = nc.dram_tensor("kmT", [BPC, KKP, 1], FP32, kind="ExternalInput")
    b2c = nc.dram_tensor("b2c", [128, 1], FP32, kind="ExternalInput")
    y = nc.dram_tensor("y", [BPC, QP32, H], FP32, kind="ExternalOutput")

    with tile.TileContext(nc) as tc:
        with (
            tc.tile_pool(name="const", bufs=1) as cp,
            tc.tile_pool(name="acts", bufs=1) as ap,
            tc.tile_pool(name="hid", bufs=28) as hp,
            tc.tile_pool(name="small", bufs=2) as sp,
            tc.tile_pool(name="psA", bufs=2, space=bass.MemorySpace.PSUM) as psA,
            tc.tile_pool(name="psS", bufs=1, space=bass.MemorySpace.PSUM) as psS,
            tc.tile_pool(name="psB", bufs=2, space=bass.MemorySpace.PSUM) as psB,
        ):
            # ---------------- constants / inputs to SBUF ----------------
            # DMA priority: critical-path first (w1k/kT feed the first
            # projections), tiny tensors next, bulky late-use last (vaug).
            w1q, w1k = {}, {}
            w1t = {}
            for dc in range(NDC):
                tk = cp.tile([128, HM], BF16, name=f"w1ks{dc}", tag=f"w1ks{dc}")
                nc.sync.dma_start(tk[:], w1.ap()[H + dc * 128 : H + (dc + 1) * 128, :])
                w1t["k", dc] = tk
            kT_sb, qT_sb, va_sb = {}, {}, {}
            for b in range(BPC):
                for dc in range(NDC):
                    t = cp.tile([128, KKP], BF16, name=f"kT{b}{dc}", tag=f"kT{b}{dc}")
                    nc.sync.dma_start(t[:], kT.ap()[b, dc * 128 : (dc + 1) * 128, :])
                    kT_sb[b, dc] = t
            for dc in range(NDC):
                tq = cp.tile([128, HM], BF16, name=f"w1qs{dc}", tag=f"w1qs{dc}")
                nc.sync.dma_start(tq[:], w1.ap()[dc * 128 : (dc + 1) * 128, :])
                w1t["q", dc] = tq
            for b in range(BPC):
                for dc in range(NDC):
                    t = cp.tile([128, QNP], BF16, name=f"qT{b}{dc}", tag=f"qT{b}{dc}")
                    nc.sync.dma_start(t[:], qT.ap()[b, dc * 128 : (dc + 1) * 128, :])
                    qT_sb[b, dc] = t
            for dc in range(NDC):
                for hb in range(NHB):
                    w1q[dc, hb] = w1t["q", dc][:, hb * 128 : (hb + 1) * 128]
                    w1k[dc, hb] = w1t["k", dc][:, hb * 128 : (hb + 1) * 128]
            b1_sb = {}
            for hb in range(NHB):
                t = cp.tile([128, 1], FP32, name=f"b1{hb}", tag=f"b1{hb}")
                nc.sync.dma_start(t[:], b1T.ap()[hb])
                b1_sb[hb] = t
            zm = cp.tile([128, NHB * 64], BF16, name="zm", tag="zm")
            nc.sync.dma_start(zm[:], zmr.ap())
            km_sb = {}
            for b in range(BPC):
                for kc in range(KC):
                    kw = min(128, KKP - kc * 128)
                    t = cp.tile([128, 1], FP32, name=f"km{b}{kc}", tag=f"km{b}{kc}")
                    nc.sync.dma_start(t[:kw, :], kmT.ap()[b, kc * 128 : kc * 128 + kw, :])
                    km_sb[b, kc] = t
            b2_sb = cp.tile([128, 1], FP32, name="b2", tag="b2")
            nc.sync.dma_start(b2_sb[:], b2c.ap())
            ident = cp.tile([128, 128], FP32, name="ident", tag="ident")
            make_identity(nc, ident[:])
            for b in range(BPC):
                for kc in range(KC):
                    kw = min(128, KKP - kc * 128)
                    t = cp.tile([128, VA], FP32, name=f"va{b}{kc}", tag=f"va{b}{kc}")
                    nc.sync.dma_start(t[:kw, :], vaug.ap()[b, kc * 128 : kc * 128 + kw, :])
                    va_sb[b, kc] = t

            # ---------------- stage 0: projections ----------------
            IDENT_F = mybir.ActivationFunctionType.Identity
            qpT, kpB = {}, {}
            for hb in range(NHB):
                for b in range(BPC):
                    ps = psA.tile([128, KKP], FP32, name="proj", tag="proj")
                    for dc in range(NDC):
                        nc.tensor.matmul(
                            ps[:, :QNP],
                            w1q[dc, hb],
                            qT_sb[b, dc][:],
                            start=(dc == 0),
                            stop=(dc == NDC - 1),
                        )
                    t = ap.tile([128, QNP], FP32, name=f"qpT{b}{hb}", tag=f"qpT{b}{hb}")
                    nc.scalar.activation(t[:], ps[:, :QNP], IDENT_F, bias=b1_sb[hb][:, 0:1])
                    qpT[b, hb] = t
                    ps2 = psA.tile([128, KKP], FP32, name="proj", tag="proj")
                    for dc in range(NDC):
                        nc.tensor.matmul(
                            ps2[:],
                            w1k[dc, hb],
                            kT_sb[b, dc][:],
                            start=(dc == 0),
                            stop=(dc == NDC - 1),
                        )
                    t2 = ap.tile([128, KKP], BF16, name=f"kpB{b}{hb}", tag=f"kpB{b}{hb}")
                    nc.scalar.activation(t2[:], ps2[:], IDENT_F, bias=b1_sb[hb][:, 0:1])
                    kpB[b, hb] = t2

            # ---------------- pair stage + score reduce ----------------
            scores = psS.tile([128, W2K], FP32, name="scps", tag="scps")
            cnt = 0
            for hb in range(NHB):
                for j in range(32):
                    w_ap = zm[:, hb * 64 + (31 - j) : hb * 64 + (63 - j)]
                    for cg in range(CGC):
                        q = cg * 32 + j
                        if q >= QNP:
                            continue
                        hid = hp.tile([128, W2K], BF16, name="hid", tag="hid")
                        for b in range(BPC):
                            qcol = qpT[b, hb][:, q : q + 1]
                            dst = hid[:, b * KKP : (b + 1) * KKP]
                            if cnt % 15 in ACT_PAT:
                                nc.scalar.activation(
                                    dst, kpB[b, hb][:], RELU, bias=qcol, scale=1.0
                                )
                            else:
                                nc.vector.tensor_scalar(
                                    dst, kpB[b, hb][:], qcol, 0.0, ADD, MAX
                                )
                            cnt += 1
                        jlast = min(31, QNP - 1 - cg * 32)
                        nc.tensor.matmul(
                            scores[cg * 32 : (cg + 1) * 32, :],
                            w_ap,
                            hid[:],
                            start=(hb == 0 and j == 0),
                            stop=(hb == NHB - 1 and j == jlast),
                            tile_position=(0, cg * 32),
                        )

            # ---------------- exp / transpose / mask / value matmul ----------------
            A = ap.tile([128, W2K], FP32, name="Aexp", tag="Aexp")
            for b in range(BPC):
                nc.scalar.activation(
                    A[:QP32, b * KKP : (b + 1) * KKP],
                    scores[:QP32, b * KKP : (b + 1) * KKP],
                    EXP,
                    bias=b2_sb[:QP32, 0:1],
                    scale=1.0,
                )
                AT = {}
                for kc in range(KC):
                    kw = min(128, KKP - kc * 128)
                    pst = psB.tile([128, QP32], FP32, name="trps", tag="trps")
                    nc.tensor.transpose(
                        pst[:kw, :],
                        A[:QP32, b * KKP + kc * 128 : b * KKP + kc * 128 + kw],
                        ident[:QP32, :QP32],
                    )
                    at = ap.tile([128, QP32], FP32, name=f"AT{b}{kc}", tag=f"AT{b}{kc}")
                    nc.scalar.activation(at[:kw, :], pst[:kw, :], IDENT_F, scale=km_sb[b, kc][:kw, 0:1])
                    AT[kc] = at
                pso = psB.tile([128, VA], FP32, name="oun", tag="oun")
                for kc in range(KC):
                    kw = min(128, KKP - kc * 128)
                    nc.tensor.matmul(
                        pso[:QP32, :],
                        AT[kc][:kw, :],
                        va_sb[b, kc][:kw, :],
                        start=(kc == 0),
                        stop=(kc == KC - 1),
                    )
                asum = sp.tile([128, 1], FP32, name="asum", tag="asum")
                nc.vector.tensor_scalar_max(asum[:QP32, :], pso[:QP32, H : H + 1], 2e-15)
                rec = sp.tile([128, 1], FP32, name="rec", tag="rec")
                nc.vector.reciprocal(rec[:QP32, :], asum[:QP32, :])
                outt = ap.tile([128, H], FP32, name=f"out{b}", tag=f"out{b}")
                nc.scalar.activation(outt[:QP32, :], pso[:QP32, 0:H], IDENT_F, scale=rec[:QP32, 0:1])
                nc.sync.dma_start(y.ap()[b], outt[:QP32, :])

    nc.compile()
    _cache[key] = nc
    return nc


def _round_up(x, m):
    return ((max(x, 1) + m - 1) // m) * m


def _prep(query, key, value, q_mask, k_mask, W1, b1, W2, b2):
    query = np.asarray(query, np.float32)
    key = np.asarray(key, np.float32)
    value = np.asarray(value, np.float32)
    q_mask = np.asarray(q_mask, np.float32)
    k_mask = np.asarray(k_mask, np.float32)
    W1 = np.ascontiguousarray(np.asarray(W1, ml_dtypes.bfloat16))
    b1 = np.asarray(b1, np.float32)
    W2 = np.asarray(W2, np.float32)
    b2 = np.asarray(b2, np.float32)

    q_idx = [np.nonzero(q_mask[i] != 0)[0] for i in range(B)]
    k_idx = [np.nonzero(k_mask[i] != 0)[0] for i in range(B)]
    QNP = _round_up(max(len(ix) for ix in q_idx), 4)
    KKP = _round_up(max(len(ix) for ix in k_idx), 8)

    # compacted, padded host-side arrays
    qTc = np.zeros((B, H, QNP), ml_dtypes.bfloat16)
    kTc = np.zeros((B, H, KKP), ml_dtypes.bfloat16)
    vaug = np.zeros((B, KKP, VA), np.float32)
    kmc = np.zeros((B, KKP, 1), np.float32)
    for i in range(B):
        qi, ki = q_idx[i], k_idx[i]
        if len(qi):
            qTc[i, :, : len(qi)] = query[i, qi, :].T.astype(ml_dtypes.bfloat16)
        if len(ki):
            kTc[i, :, : len(ki)] = key[i, ki, :].T.astype(ml_dtypes.bfloat16)
            vaug[i, : len(ki), :H] = value[i, ki, :]
            vaug[i, : len(ki), H] = 1.0
            kmc[i, : len(ki), 0] = 1.0

    b1T = np.ascontiguousarray(b1.reshape(NHB, 128, 1))
    zmr = np.zeros((128, NHB * 64), np.float32)
    for hb in range(NHB):
        zmr[:, hb * 64 + 31] = W2[hb * 128 : (hb + 1) * 128, 0]
    zmr = zmr.astype(ml_dtypes.bfloat16)
    b2c = np.full((128, 1), float(b2[0]), np.float32)

    in_maps = []
    for c in range(N_CORES):
        sl = slice(BPC * c, BPC * (c + 1))
        in_maps.append(
            {
                "qT": np.ascontiguousarray(qTc[sl]),
                "kT": np.ascontiguousarray(kTc[sl]),
                "vaug": np.ascontiguousarray(vaug[sl]),
                "w1": W1,
                "b1T": b1T,
                "zmr": zmr,
                "kmT": np.ascontiguousarray(kmc[sl]),
                "b2c": b2c,
            }
        )
    return in_maps, q_idx, QNP, KKP


def kernel(query, key, value, q_mask, k_mask, W1, b1, W2, b2):
    in_maps, q_idx, QNP, KKP = _prep(
        query, key, value, q_mask, k_mask, W1, b1, W2, b2
    )
    nc = _build(QNP, KKP)
    res = run_bass_kernel_spmd(nc, in_maps, core_ids=list(range(N_CORES)))
    out = np.zeros((B, S1, H), np.float32)
    for c in range(N_CORES):
        yv = res.results[c]["y"]
        for b in range(BPC):
            gi = BPC * c + b
            qi = q_idx[gi]
            if len(qi):
                out[gi, qi, :] = yv[b, : len(qi), :]
    return out


def traced_single_core(query, key, value, q_mask, k_mask, W1, b1, W2, b2, core=0):
    """Run one core's share with NTFF tracing; returns (out, exec_time_ns)."""
    in_maps, q_idx, QNP, KKP = _prep(
        query, key, value, q_mask, k_mask, W1, b1, W2, b2
    )
    nc = _build(QNP, KKP)
    res = run_bass_kernel_spmd(nc, [in_maps[core]], core_ids=[0], trace=True)
    out = np.zeros((BPC, S1, H), np.float32)
    yv = res.results[0]["y"]
    for b in range(BPC):
        gi = BPC * core + b
        qi = q_idx[gi]
        if len(qi):
            out[b, qi, :] = yv[b, : len(qi), :]
    return out, res.exec_time_ns


# revision 7
# speedup vs baseline: 1.2841x; 1.0938x over previous
"""Trainium2 Bass kernel for nn_AttentionLayer (additive/Bahdanau-style attention).

Reference computation:
  qp = query @ W1[:H] ; kp = key @ W1[H:]          # [B,S1,HM], [B,S2,HM]
  hid = relu(qp[:,:,None,:] + kp[:,None,:,:] + b1)  # [B,S1,S2,HM]
  scores = hid @ W2 + b2                            # [B,S1,S2]
  A = where(qmask*kmask==0, 0, exp(scores))
  out = (A / clip(A.sum(-1), 2e-15)) @ value        # [B,S1,H]

Sharding: data-parallel over batch, 2 batches per core on 8 cores.

Sparsity: masked q rows produce all-zero output rows and masked k columns
contribute exactly zero to every sum, so the host compacts both dimensions
per batch (keeping only mask==1 rows, zero-padded up to a common per-call
capacity) and scatters the result back. The device still applies the k-mask
(padding lanes carry mask 0), so results are exact.

Device mapping (per core, 2 batches):
  stage0 (PE):   qpT[h,q], kpT[h,k] projections (+b1), kpT cast bf16
  pair  (DVE/ACT): hidT[h,k] = relu(kpT_hb + qpT_hb[:,q]) per (b,q,hb), one
                 fused per-partition-scalar op; both batches share one wide
                 [128, 2*KKP] tile so the PE consumes them in one matmul
  scores (PE):   matmul with zero-padded-column weights Z(hb,j)[128,32]
                 (col j = W2 h-block) -> scores row lands on PSUM partition
                 q=32*cg+j (tile_position col groups), fp32 accumulation
  post:          Exp(+b2) over both batches at once; PE transpose -> A_T[k,q];
                 k_mask applied on PSUM evict; out_un = A_T.T @ [value | ones]
                 (ones column = row-sum); out = out_un * 1/clip(rowsum).
"""

import os
import sys

import numpy as np

for _p in ("/opt/trn_rl_repo",):
    if os.path.isdir(_p) and _p not in sys.path:
        sys.path.insert(0, _p)

import ml_dtypes  # noqa: E402
import concourse.bass as bass  # noqa: E402
import concourse.mybir as mybir  # noqa: E402
import concourse.tile as tile  # noqa: E402
from concourse import bacc  # noqa: E402
from concourse.bass_utils import run_bass_kernel_spmd  # noqa: E402
from concourse.masks import make_identity  # noqa: E402

B, S1, S2, H, HM = 16, 128, 256, 256, 512
N_CORES = 8
BPC = B // N_CORES  # batches per core
NHB = HM // 128  # h blocks
NDC = H // 128  # d chunks (projection contraction)
VA = H + 2  # value dims + ones column + pad
ACT_PAT = (0, 3, 6, 9, 12)  # which cnt%16 residues run on the scalar engine

FP32 = mybir.dt.float32
BF16 = mybir.dt.bfloat16
ADD = mybir.AluOpType.add
MAX = mybir.AluOpType.max
RELU = mybir.ActivationFunctionType.Relu
EXP = mybir.ActivationFunctionType.Exp

_cache: dict = {}


def _build(QNP, KKP):
    """Build + compile the per-core program for q-capacity QNP, k-capacity KKP."""
    key = (QNP, KKP)
    if key in _cache:
        return _cache[key]

    CGC = (QNP + 31) // 32  # col groups used
    QP32 = CGC * 32  # partition extent of the scores region
    KC = (KKP + 127) // 128  # k chunks for transpose/value
    W2K = 2 * KKP  # wide (both batches) k extent

    nc = bacc.Bacc("TRN2", target_bir_lowering=False, debug=False, num_devices=1)

    KC_ = (KKP + 127) // 128
    qT = nc.dram_tensor("qT", [128, BPC * NDC * QNP], BF16, kind="ExternalInput")
    kT = nc.dram_tensor("kT", [128, BPC * NDC * KKP], BF16, kind="ExternalInput")
    vaug = nc.dram_tensor("vaug", [128, BPC * KC_ * VA], FP32, kind="ExternalInput")
    w1 = nc.dram_tensor("w1", [2 * H, HM], BF16, kind="ExternalInput")
    b1T = nc.dram_tensor("b1T", [128, NHB], FP32, kind="ExternalInput")
    zmr = nc.dram_tensor("zmr", [128, NHB * 64], BF16, kind="ExternalInput")
    kmT = nc.dram_tensor("kmT", [128, BPC * KC_], FP32, kind="ExternalInput")
    b2c = nc.dram_tensor("b2c", [128, 1], FP32, kind="ExternalInput")
    y = nc.dram_tensor("y", [BPC, QP32, H], FP32, kind="ExternalOutput")

    with tile.TileContext(nc) as tc:
        with (
            tc.tile_pool(name="const", bufs=1) as cp,
            tc.tile_pool(name="acts", bufs=1) as ap,
            tc.tile_pool(name="hid", bufs=28) as hp,
            tc.tile_pool(name="small", bufs=2) as sp,
            tc.tile_pool(name="psA", bufs=2, space=bass.MemorySpace.PSUM) as psA,
            tc.tile_pool(name="psS", bufs=1, space=bass.MemorySpace.PSUM) as psS,
            tc.tile_pool(name="psB", bufs=2, space=bass.MemorySpace.PSUM) as psB,
        ):
            # ---------------- constants / inputs to SBUF ----------------
            # One wide DMA per logical input (HWDGE issue on the sync
            # sequencer is ~0.6us per dma_start; few big beats many small).
            # Order = critical path: w1k/kT feed the first projections.
            w1q, w1k = {}, {}
            w1t = {}
            for dc in range(NDC):
                tk = cp.tile([128, HM], BF16, name=f"w1ks{dc}", tag=f"w1ks{dc}")
                nc.sync.dma_start(tk[:], w1.ap()[H + dc * 128 : H + (dc + 1) * 128, :])
                w1t["k", dc] = tk
            kT_all = cp.tile([128, BPC * NDC * KKP], BF16, name="kT_all", tag="kT_all")
            nc.sync.dma_start(kT_all[:], kT.ap())
            for dc in range(NDC):
                tq = cp.tile([128, HM], BF16, name=f"w1qs{dc}", tag=f"w1qs{dc}")
                nc.sync.dma_start(tq[:], w1.ap()[dc * 128 : (dc + 1) * 128, :])
                w1t["q", dc] = tq
            qT_all = cp.tile([128, BPC * NDC * QNP], BF16, name="qT_all", tag="qT_all")
            nc.sync.dma_start(qT_all[:], qT.ap())
            b1_all = cp.tile([128, NHB], FP32, name="b1_all", tag="b1_all")
            nc.sync.dma_start(b1_all[:], b1T.ap())
            zm = cp.tile([128, NHB * 64], BF16, name="zm", tag="zm")
            nc.sync.dma_start(zm[:], zmr.ap())
            km_all = cp.tile([128, BPC * KC], FP32, name="km_all", tag="km_all")
            nc.sync.dma_start(km_all[:], kmT.ap())
            b2_sb = cp.tile([128, 1], FP32, name="b2", tag="b2")
            nc.sync.dma_start(b2_sb[:], b2c.ap())
            va_all = cp.tile([128, BPC * KC * VA], FP32, name="va_all", tag="va_all")
            nc.sync.dma_start(va_all[:], vaug.ap())
            ident = cp.tile([128, 128], FP32, name="ident", tag="ident")
            make_identity(nc, ident[:])

            kT_sb, qT_sb, va_sb, km_sb, b1_sb = {}, {}, {}, {}, {}
            for b in range(BPC):
                for dc in range(NDC):
                    blk = b * NDC + dc
                    kT_sb[b, dc] = kT_all[:, blk * KKP : (blk + 1) * KKP]
                    qT_sb[b, dc] = qT_all[:, blk * QNP : (blk + 1) * QNP]
                for kc in range(KC):
                    va_sb[b, kc] = va_all[:, (b * KC + kc) * VA : (b * KC + kc + 1) * VA]
                    km_sb[b, kc] = km_all[:, b * KC + kc : b * KC + kc + 1]
            for hb in range(NHB):
                b1_sb[hb] = b1_all[:, hb : hb + 1]
            for dc in range(NDC):
                for hb in range(NHB):
                    w1q[dc, hb] = w1t["q", dc][:, hb * 128 : (hb + 1) * 128]
                    w1k[dc, hb] = w1t["k", dc][:, hb * 128 : (hb + 1) * 128]

            # ---------------- stage 0: projections ----------------
            IDENT_F = mybir.ActivationFunctionType.Identity
            qpT, kpB = {}, {}
            for hb in range(NHB):
                for b in range(BPC):
                    ps = psA.tile([128, KKP], FP32, name="proj", tag="proj")
                    for dc in range(NDC):
                        nc.tensor.matmul(
                            ps[:, :QNP],
                            w1q[dc, hb],
                            qT_sb[b, dc],
                            start=(dc == 0),
                            stop=(dc == NDC - 1),
                        )
                    t = ap.tile([128, QNP], FP32, name=f"qpT{b}{hb}", tag=f"qpT{b}{hb}")
                    nc.scalar.activation(t[:], ps[:, :QNP], IDENT_F, bias=b1_sb[hb])
                    qpT[b, hb] = t
                    ps2 = psA.tile([128, KKP], FP32, name="proj", tag="proj")
                    for dc in range(NDC):
                        nc.tensor.matmul(
                            ps2[:],
                            w1k[dc, hb],
                            kT_sb[b, dc],
                            start=(dc == 0),
                            stop=(dc == NDC - 1),
                        )
                    t2 = ap.tile([128, KKP], BF16, name=f"kpB{b}{hb}", tag=f"kpB{b}{hb}")
                    nc.scalar.activation(t2[:], ps2[:], IDENT_F, bias=b1_sb[hb])
                    kpB[b, hb] = t2

            # ---------------- pair stage + score reduce ----------------
            scores = psS.tile([128, W2K], FP32, name="scps", tag="scps")
            cnt = 0
            for hb in range(NHB):
                for j in range(32):
                    w_ap = zm[:, hb * 64 + (31 - j) : hb * 64 + (63 - j)]
                    for cg in range(CGC):
                        q = cg * 32 + j
                        if q >= QNP:
                            continue
                        hid = hp.tile([128, W2K], BF16, name="hid", tag="hid")
                        for b in range(BPC):
                            qcol = qpT[b, hb][:, q : q + 1]
                            dst = hid[:, b * KKP : (b + 1) * KKP]
                            if cnt % 16 in ACT_PAT:
                                nc.scalar.activation(
                                    dst, kpB[b, hb][:], RELU, bias=qcol, scale=1.0
                                )
                            else:
                                nc.vector.tensor_scalar(
                                    dst, kpB[b, hb][:], qcol, 0.0, ADD, MAX
                                )
                            cnt += 1
                        jlast = min(31, QNP - 1 - cg * 32)
                        nc.tensor.matmul(
                            scores[cg * 32 : (cg + 1) * 32, :],
                            w_ap,
                            hid[:],
                            start=(hb == 0 and j == 0),
                            stop=(hb == NHB - 1 and j == jlast),
                            tile_position=(0, cg * 32),
                        )

            # ---------------- exp / transpose / mask / value matmul ----------------
            A = ap.tile([128, W2K], FP32, name="Aexp", tag="Aexp")
            for b in range(BPC):
                nc.scalar.activation(
                    A[:QP32, b * KKP : (b + 1) * KKP],
                    scores[:QP32, b * KKP : (b + 1) * KKP],
                    EXP,
                    bias=b2_sb[:QP32, 0:1],
                    scale=1.0,
                )
                AT = {}
                for kc in range(KC):
                    kw = min(128, KKP - kc * 128)
                    pst = psB.tile([128, QP32], FP32, name="trps", tag="trps")
                    nc.tensor.transpose(
                        pst[:kw, :],
                        A[:QP32, b * KKP + kc * 128 : b * KKP + kc * 128 + kw],
                        ident[:QP32, :QP32],
                    )
                    at = ap.tile([128, QP32], FP32, name=f"AT{b}{kc}", tag=f"AT{b}{kc}")
                    nc.scalar.activation(at[:kw, :], pst[:kw, :], IDENT_F, scale=km_sb[b, kc][0:kw, :])
                    AT[kc] = at
                pso = psB.tile([128, VA], FP32, name="oun", tag="oun")
                for kc in range(KC):
                    kw = min(128, KKP - kc * 128)
                    nc.tensor.matmul(
                        pso[:QP32, :],
                        AT[kc][:kw, :],
                        va_sb[b, kc][0:kw, :],
                        start=(kc == 0),
                        stop=(kc == KC - 1),
                    )
                asum = sp.tile([128, 1], FP32, name="asum", tag="asum")
                nc.vector.tensor_scalar_max(asum[:QP32, :], pso[:QP32, H : H + 1], 2e-15)
                rec = sp.tile([128, 1], FP32, name="rec", tag="rec")
                nc.vector.reciprocal(rec[:QP32, :], asum[:QP32, :])
                outt = ap.tile([128, H], FP32, name=f"out{b}", tag=f"out{b}")
                nc.scalar.activation(outt[:QP32, :], pso[:QP32, 0:H], IDENT_F, scale=rec[:QP32, 0:1])
                nc.sync.dma_start(y.ap()[b], outt[:QP32, :])

    nc.compile()
    _cache[key] = nc
    return nc


def _round_up(x, m):
    return ((max(x, 1) + m - 1) // m) * m


def _prep(query, key, value, q_mask, k_mask, W1, b1, W2, b2):
    query = np.asarray(query, np.float32)
    key = np.asarray(key, np.float32)
    value = np.asarray(value, np.float32)
    q_mask = np.asarray(q_mask, np.float32)
    k_mask = np.asarray(k_mask, np.float32)
    W1 = np.ascontiguousarray(np.asarray(W1, ml_dtypes.bfloat16))
    b1 = np.asarray(b1, np.float32)
    W2 = np.asarray(W2, np.float32)
    b2 = np.asarray(b2, np.float32)

    q_idx = [np.nonzero(q_mask[i] != 0)[0] for i in range(B)]
    k_idx = [np.nonzero(k_mask[i] != 0)[0] for i in range(B)]
    QNP = _round_up(max(len(ix) for ix in q_idx), 4)
    KKP = _round_up(max(len(ix) for ix in k_idx), 8)

    # compacted, padded host-side arrays (packed in SBUF partition layout)
    NDCl, NHBl = NDC, NHB
    KC = (KKP + 127) // 128
    qTc = np.zeros((B, H, QNP), ml_dtypes.bfloat16)
    kTc = np.zeros((B, H, KKP), ml_dtypes.bfloat16)
    vaug = np.zeros((B, KC * 128, VA), np.float32)
    kmc = np.zeros((B, KC * 128), np.float32)
    for i in range(B):
        qi, ki = q_idx[i], k_idx[i]
        if len(qi):
            qTc[i, :, : len(qi)] = query[i, qi, :].T.astype(ml_dtypes.bfloat16)
        if len(ki):
            kTc[i, :, : len(ki)] = key[i, ki, :].T.astype(ml_dtypes.bfloat16)
            vaug[i, : len(ki), :H] = value[i, ki, :]
            vaug[i, : len(ki), H] = 1.0
            kmc[i, : len(ki)] = 1.0

    zmr = np.zeros((128, NHB * 64), np.float32)
    for hb in range(NHB):
        zmr[:, hb * 64 + 31] = W2[hb * 128 : (hb + 1) * 128, 0]
    zmr = zmr.astype(ml_dtypes.bfloat16)
    b2c = np.full((128, 1), float(b2[0]), np.float32)

    in_maps = []
    for c in range(N_CORES):
        bs = [BPC * c + b for b in range(BPC)]
        qTp = np.zeros((128, BPC * NDCl * QNP), ml_dtypes.bfloat16)
        kTp = np.zeros((128, BPC * NDCl * KKP), ml_dtypes.bfloat16)
        vap = np.zeros((128, BPC * KC * VA), np.float32)
        kmp = np.zeros((128, BPC * KC), np.float32)
        for b, gi in enumerate(bs):
            for dc in range(NDCl):
                blk = b * NDCl + dc
                qTp[:, blk * QNP : (blk + 1) * QNP] = qTc[gi, dc * 128 : (dc + 1) * 128, :]
                kTp[:, blk * KKP : (blk + 1) * KKP] = kTc[gi, dc * 128 : (dc + 1) * 128, :]
            for kc in range(KC):
                vap[:, (b * KC + kc) * VA : (b * KC + kc + 1) * VA] = vaug[
                    gi, kc * 128 : (kc + 1) * 128, :
                ]
                kmp[:, b * KC + kc] = kmc[gi, kc * 128 : (kc + 1) * 128]
        in_maps.append(
            {
                "qT": qTp,
                "kT": kTp,
                "vaug": vap,
                "w1": W1,
                "b1T": np.ascontiguousarray(
                    b1.reshape(NHBl, 128).T.astype(np.float32)
                ),
                "zmr": zmr,
                "kmT": kmp,
                "b2c": b2c,
            }
        )
    return in_maps, q_idx, QNP, KKP


def kernel(query, key, value, q_mask, k_mask, W1, b1, W2, b2):
    in_maps, q_idx, QNP, KKP = _prep(
        query, key, value, q_mask, k_mask, W1, b1, W2, b2
    )
    nc = _build(QNP, KKP)
    res = run_bass_kernel_spmd(nc, in_maps, core_ids=list(range(N_CORES)))
    out = np.zeros((B, S1, H), np.float32)
    for c in range(N_CORES):
        yv = res.results[c]["y"]
        for b in range(BPC):
            gi = BPC * c + b
            qi = q_idx[gi]
            if len(qi):
                out[gi, qi, :] = yv[b, : len(qi), :]
    return out


def traced_single_core(query, key, value, q_mask, k_mask, W1, b1, W2, b2, core=0):
    """Run one core's share with NTFF tracing; returns (out, exec_time_ns)."""
    in_maps, q_idx, QNP, KKP = _prep(
        query, key, value, q_mask, k_mask, W1, b1, W2, b2
    )
    nc = _build(QNP, KKP)
    res = run_bass_kernel_spmd(nc, [in_maps[core]], core_ids=[0], trace=True)
    out = np.zeros((BPC, S1, H), np.float32)
    yv = res.results[0]["y"]
    for b in range(BPC):
        gi = BPC * core + b
        qi = q_idx[gi]
        if len(qi):
            out[b, qi, :] = yv[b, : len(qi), :]
    return out, res.exec_time_ns


# revision 8
# speedup vs baseline: 1.4242x; 1.1091x over previous
"""Trainium2 Bass kernel for nn_AttentionLayer (additive/Bahdanau-style attention).

Reference computation:
  qp = query @ W1[:H] ; kp = key @ W1[H:]          # [B,S1,HM], [B,S2,HM]
  hid = relu(qp[:,:,None,:] + kp[:,None,:,:] + b1)  # [B,S1,S2,HM]
  scores = hid @ W2 + b2                            # [B,S1,S2]
  A = where(qmask*kmask==0, 0, exp(scores))
  out = (A / clip(A.sum(-1), 2e-15)) @ value        # [B,S1,H]

Sharding: data-parallel over batch, 2 batches per core on 8 cores.

Sparsity: masked q rows produce all-zero output rows and masked k columns
contribute exactly zero to every sum, so the host compacts both dimensions
per batch (keeping only mask==1 rows, zero-padded to per-SLOT capacities)
and scatters the result back. Batches are sorted by kept-q count and paired
big-with-small across the two per-core slots, so slot capacities hug the
actual counts. The device still applies the k-mask (padding lanes carry
mask 0), so results are exact up to bf16 rounding of W1/query/key/hid.

Device mapping (per core, 2 batch slots):
  stage0 (PE):   qpT[h,q], kpT[h,k] projections (+b1 via ACT Identity evict)
  pair  (DVE/ACT): hidT[h,k] = relu(kpT_hb + qpT_hb[:,q]) per (slot,q,hb) as
                 one fused per-partition-scalar op; both slots write one wide
                 [128, K0+K1] tile consumed by a single PE matmul
  scores (PE):   matmul with sliding-window weights Z(hb,j) = zmr[:,31-j:63-j]
                 (W2 h-block parked at column 31) -> scores row lands on PSUM
                 partition q=32*cg+j via tile_position col groups, fp32 accum
  post (per slot): Exp(+b2); PE transpose -> A_T[k,q]; k_mask on PSUM evict;
                 out_un = A_T.T @ [value | ones] (ones column = row-sum);
                 out = out_un * 1/clip(rowsum, 2e-15) per-partition.
"""

import os
import sys

import numpy as np

for _p in ("/opt/trn_rl_repo",):
    if os.path.isdir(_p) and _p not in sys.path:
        sys.path.insert(0, _p)

import ml_dtypes  # noqa: E402
import concourse.bass as bass  # noqa: E402
import concourse.mybir as mybir  # noqa: E402
import concourse.tile as tile  # noqa: E402
from concourse import bacc  # noqa: E402
from concourse.bass_utils import run_bass_kernel_spmd  # noqa: E402
from concourse.masks import make_identity  # noqa: E402

B, S1, S2, H, HM = 16, 128, 256, 256, 512
N_CORES = 8
BPC = B // N_CORES  # batch slots per core
NHB = HM // 128  # h blocks
NDC = H // 128  # d chunks (projection contraction)
VA = H + 2  # value dims + ones column + pad
ACT_PAT = (0, 3, 6, 9, 12, 15, 18, 21, 24, 27, 30)  # cnt%32 residues on ACT

FP32 = mybir.dt.float32
BF16 = mybir.dt.bfloat16
ADD = mybir.AluOpType.add
MAX = mybir.AluOpType.max
RELU = mybir.ActivationFunctionType.Relu
EXP = mybir.ActivationFunctionType.Exp
IDENT_F = mybir.ActivationFunctionType.Identity

_cache: dict = {}


def _build(QN, KK):
    """Build + compile the per-core program.

    QN/KK: per-slot q and k capacities, e.g. QN=(80, 64), KK=(144, 136).
    """
    ck = (QN, KK)
    if ck in _cache:
        return _cache[ck]

    CGC = [(q + 31) // 32 for q in QN]  # col groups per slot
    QP32 = [c * 32 for c in CGC]
    KC = [(k + 127) // 128 for k in KK]  # k chunks per slot
    KOFF = [0, KK[0]]  # slot column offsets in wide tiles
    KW = KK[0] + KK[1]  # wide k extent
    KCT = sum(KC)
    QTW = NDC * (QN[0] + QN[1])  # packed qT width
    KTW = NDC * (KK[0] + KK[1])  # packed kT width

    nc = bacc.Bacc("TRN2", target_bir_lowering=False, debug=False, num_devices=1)

    qT = nc.dram_tensor("qT", [128, QTW], BF16, kind="ExternalInput")
    kT = nc.dram_tensor("kT", [128, KTW], BF16, kind="ExternalInput")
    vaug = nc.dram_tensor("vaug", [128, KCT * VA], FP32, kind="ExternalInput")
    w1 = nc.dram_tensor("w1", [2 * H, HM], BF16, kind="ExternalInput")
    b1T = nc.dram_tensor("b1T", [128, NHB], FP32, kind="ExternalInput")
    zmr = nc.dram_tensor("zmr", [128, NHB * 64], BF16, kind="ExternalInput")
    kmT = nc.dram_tensor("kmT", [128, KCT], FP32, kind="ExternalInput")
    b2c = nc.dram_tensor("b2c", [128, 1], FP32, kind="ExternalInput")
    y = nc.dram_tensor("y", [BPC, QP32[0], H], FP32, kind="ExternalOutput")

    with tile.TileContext(nc) as tc:
        with (
            tc.tile_pool(name="const", bufs=1) as cp,
            tc.tile_pool(name="acts", bufs=1) as ap,
            tc.tile_pool(name="hid", bufs=28) as hp,
            tc.tile_pool(name="small", bufs=2) as sp,
            tc.tile_pool(name="psA", bufs=2, space=bass.MemorySpace.PSUM) as psA,
            tc.tile_pool(name="psS", bufs=1, space=bass.MemorySpace.PSUM) as psS,
            tc.tile_pool(name="psB", bufs=2, space=bass.MemorySpace.PSUM) as psB,
        ):
            # ---------------- inputs to SBUF ----------------
            # Few wide DMAs (HWDGE issue is ~0.6us each on the sync
            # sequencer); critical path (w1k/kT -> first projections) first,
            # big loads split in halves so two queues stream in parallel.
            w1t = {}
            for dc in range(NDC):
                tk = cp.tile([128, HM], BF16, name=f"w1ks{dc}", tag=f"w1ks{dc}")
                nc.sync.dma_start(tk[:], w1.ap()[H + dc * 128 : H + (dc + 1) * 128, :])
                w1t["k", dc] = tk
            kT_all = cp.tile([128, KTW], BF16, name="kT_all", tag="kT_all")
            hw = KTW // 2
            nc.sync.dma_start(kT_all[:, :hw], kT.ap()[:, :hw])
            nc.sync.dma_start(kT_all[:, hw:], kT.ap()[:, hw:])
            for dc in range(NDC):
                tq = cp.tile([128, HM], BF16, name=f"w1qs{dc}", tag=f"w1qs{dc}")
                nc.sync.dma_start(tq[:], w1.ap()[dc * 128 : (dc + 1) * 128, :])
                w1t["q", dc] = tq
            qT_all = cp.tile([128, QTW], BF16, name="qT_all", tag="qT_all")
            nc.sync.dma_start(qT_all[:], qT.ap())
            b1_all = cp.tile([128, NHB], FP32, name="b1_all", tag="b1_all")
            nc.sync.dma_start(b1_all[:], b1T.ap())
            zm = cp.tile([128, NHB * 64], BF16, name="zm", tag="zm")
            nc.sync.dma_start(zm[:], zmr.ap())
            km_all = cp.tile([128, KCT], FP32, name="km_all", tag="km_all")
            nc.sync.dma_start(km_all[:], kmT.ap())
            b2_sb = cp.tile([128, 1], FP32, name="b2", tag="b2")
            nc.sync.dma_start(b2_sb[:], b2c.ap())
            va_all = cp.tile([128, KCT * VA], FP32, name="va_all", tag="va_all")
            vw = (KCT * VA) // 2
            nc.sync.dma_start(va_all[:, :vw], vaug.ap()[:, :vw])
            nc.sync.dma_start(va_all[:, vw:], vaug.ap()[:, vw:])
            ident = cp.tile([128, 128], FP32, name="ident", tag="ident")
            make_identity(nc, ident[:])

            w1q, w1k, qT_sb, kT_sb, va_sb, km_sb, b1_sb = {}, {}, {}, {}, {}, {}, {}
            qoff = koff = 0
            for b in range(BPC):
                for dc in range(NDC):
                    kT_sb[b, dc] = kT_all[:, koff : koff + KK[b]]
                    qT_sb[b, dc] = qT_all[:, qoff : qoff + QN[b]]
                    qoff += QN[b]
                    koff += KK[b]
            coff = 0
            for b in range(BPC):
                for kc in range(KC[b]):
                    va_sb[b, kc] = va_all[:, coff * VA : (coff + 1) * VA]
                    km_sb[b, kc] = km_all[:, coff : coff + 1]
                    coff += 1
            for hb in range(NHB):
                b1_sb[hb] = b1_all[:, hb : hb + 1]
            for dc in range(NDC):
                for hb in range(NHB):
                    w1q[dc, hb] = w1t["q", dc][:, hb * 128 : (hb + 1) * 128]
                    w1k[dc, hb] = w1t["k", dc][:, hb * 128 : (hb + 1) * 128]

            # ---------------- stage 0: projections ----------------
            qpT, kpB = {}, {}
            for hb in range(NHB):
                for b in range(BPC):
                    ps = psA.tile([128, max(KK)], FP32, name="proj", tag="proj")
                    for dc in range(NDC):
                        nc.tensor.matmul(
                            ps[:, : QN[b]],
                            w1q[dc, hb],
                            qT_sb[b, dc],
                            start=(dc == 0),
                            stop=(dc == NDC - 1),
                        )
                    t = ap.tile([128, QN[b]], FP32, name=f"qpT{b}{hb}", tag=f"qpT{b}{hb}")
                    nc.scalar.activation(t[:], ps[:, : QN[b]], IDENT_F, bias=b1_sb[hb])
                    qpT[b, hb] = t
                    ps2 = psA.tile([128, max(KK)], FP32, name="proj", tag="proj")
                    for dc in range(NDC):
                        nc.tensor.matmul(
                            ps2[:, : KK[b]],
                            w1k[dc, hb],
                            kT_sb[b, dc],
                            start=(dc == 0),
                            stop=(dc == NDC - 1),
                        )
                    t2 = ap.tile([128, KK[b]], BF16, name=f"kpB{b}{hb}", tag=f"kpB{b}{hb}")
                    nc.scalar.activation(t2[:], ps2[:, : KK[b]], IDENT_F, bias=b1_sb[hb])
                    kpB[b, hb] = t2

            # ---------------- pair stage + score reduce ----------------
            scores = psS.tile([128, KW], FP32, name="scps", tag="scps")
            cnt = 0
            for hb in range(NHB):
                for j in range(32):
                    w_ap = zm[:, hb * 64 + (31 - j) : hb * 64 + (63 - j)]
                    for cg in range(CGC[0]):
                        q = cg * 32 + j
                        if q >= QN[0]:
                            continue
                        hid = hp.tile([128, KW], BF16, name="hid", tag="hid")
                        nw = KK[0]
                        for b in range(BPC):
                            if q >= QN[b]:
                                continue
                            nw = KOFF[b] + KK[b]
                            qcol = qpT[b, hb][:, q : q + 1]
                            dst = hid[:, KOFF[b] : KOFF[b] + KK[b]]
                            if cnt % 32 in ACT_PAT:
                                nc.scalar.activation(
                                    dst, kpB[b, hb][:], RELU, bias=qcol, scale=1.0
                                )
                            else:
                                nc.vector.tensor_scalar(
                                    dst, kpB[b, hb][:], qcol, 0.0, ADD, MAX
                                )
                            cnt += 1
                        jlast = min(31, QN[0] - 1 - cg * 32)
                        nc.tensor.matmul(
                            scores[cg * 32 : (cg + 1) * 32, :nw],
                            w_ap,
                            hid[:, :nw],
                            start=(hb == 0 and j == 0),
                            stop=(hb == NHB - 1 and j == jlast),
                            tile_position=(0, cg * 32),
                        )

            # ---------------- per-slot: exp / transpose / mask / value ----------------
            for b in range(BPC):
                qp32 = QP32[b]
                A = ap.tile([128, KK[b]], FP32, name=f"Aexp{b}", tag=f"Aexp{b}")
                nc.scalar.activation(
                    A[:qp32, :],
                    scores[:qp32, KOFF[b] : KOFF[b] + KK[b]],
                    EXP,
                    bias=b2_sb[:qp32, 0:1],
                    scale=1.0,
                )
                AT = {}
                for kc in range(KC[b]):
                    kw = min(128, KK[b] - kc * 128)
                    pst = psB.tile([128, qp32], FP32, name="trps", tag="trps")
                    nc.tensor.transpose(
                        pst[:kw, :],
                        A[:qp32, kc * 128 : kc * 128 + kw],
                        ident[:qp32, :qp32],
                    )
                    at = ap.tile([128, qp32], FP32, name=f"AT{b}{kc}", tag=f"AT{b}{kc}")
                    nc.scalar.activation(
                        at[:kw, :], pst[:kw, :], IDENT_F, scale=km_sb[b, kc][0:kw, :]
                    )
                    AT[kc] = at
                pso = psB.tile([128, VA], FP32, name="oun", tag="oun")
                for kc in range(KC[b]):
                    kw = min(128, KK[b] - kc * 128)
                    nc.tensor.matmul(
                        pso[:qp32, :],
                        AT[kc][:kw, :],
                        va_sb[b, kc][0:kw, :],
                        start=(kc == 0),
                        stop=(kc == KC[b] - 1),
                    )
                asum = sp.tile([128, 1], FP32, name="asum", tag="asum")
                nc.vector.tensor_scalar_max(asum[:qp32, :], pso[:qp32, H : H + 1], 2e-15)
                rec = sp.tile([128, 1], FP32, name="rec", tag="rec")
                nc.vector.reciprocal(rec[:qp32, :], asum[:qp32, :])
                outt = ap.tile([128, H], FP32, name=f"out{b}", tag=f"out{b}")
                nc.scalar.activation(
                    outt[:qp32, :], pso[:qp32, 0:H], IDENT_F, scale=rec[:qp32, 0:1]
                )
                nc.sync.dma_start(y.ap()[b, 0:qp32, :], outt[:qp32, :])

    nc.compile()
    _cache[ck] = nc
    return nc


def _r(x, m):
    return ((max(int(x), 1) + m - 1) // m) * m


def _prep(query, key, value, q_mask, k_mask, W1, b1, W2, b2):
    query = np.asarray(query, np.float32)
    key = np.asarray(key, np.float32)
    value = np.asarray(value, np.float32)
    q_mask = np.asarray(q_mask, np.float32)
    k_mask = np.asarray(k_mask, np.float32)
    W1 = np.ascontiguousarray(np.asarray(W1, ml_dtypes.bfloat16))
    b1 = np.asarray(b1, np.float32)
    W2 = np.asarray(W2, np.float32)
    b2 = np.asarray(b2, np.float32)

    q_idx = [np.nonzero(q_mask[i] != 0)[0] for i in range(B)]
    k_idx = [np.nonzero(k_mask[i] != 0)[0] for i in range(B)]
    qn = np.array([len(ix) for ix in q_idx])

    # Slot assignment: sort by kept-q count; 8 largest -> slot 0, rest -> slot 1.
    order = np.argsort(-qn, kind="stable")
    slot_batches = [list(order[:N_CORES]), list(order[N_CORES:])]
    QN = tuple(_r(max(len(q_idx[i]) for i in slot_batches[s]), 4) for s in range(BPC))
    KK = tuple(_r(max(len(k_idx[i]) for i in slot_batches[s]), 8) for s in range(BPC))
    KC = [(k + 127) // 128 for k in KK]
    KCT = sum(KC)

    b1T = np.ascontiguousarray(b1.reshape(NHB, 128).T.astype(np.float32))
    zmr = np.zeros((128, NHB * 64), np.float32)
    for hb in range(NHB):
        zmr[:, hb * 64 + 31] = W2[hb * 128 : (hb + 1) * 128, 0]
    zmr = zmr.astype(ml_dtypes.bfloat16)
    b2c = np.full((128, 1), float(b2[0]), np.float32)

    assign = {}  # (core, slot) -> global batch idx
    in_maps = []
    QTW = NDC * (QN[0] + QN[1])
    KTW = NDC * (KK[0] + KK[1])
    for c in range(N_CORES):
        qTp = np.zeros((128, QTW), ml_dtypes.bfloat16)
        kTp = np.zeros((128, KTW), ml_dtypes.bfloat16)
        vap = np.zeros((128, KCT * VA), np.float32)
        kmp = np.zeros((128, KCT), np.float32)
        qoff = koff = coff = 0
        for s in range(BPC):
            gi = slot_batches[s][c]
            assign[c, s] = gi
            qi, ki = q_idx[gi], k_idx[gi]
            for dc in range(NDC):
                if len(qi):
                    qTp[:, qoff : qoff + len(qi)] = query[
                        gi, qi, dc * 128 : (dc + 1) * 128
                    ].T.astype(ml_dtypes.bfloat16)
                if len(ki):
                    kTp[:, koff : koff + len(ki)] = key[
                        gi, ki, dc * 128 : (dc + 1) * 128
                    ].T.astype(ml_dtypes.bfloat16)
                qoff += QN[s]
                koff += KK[s]
            for kc in range(KC[s]):
                lo, hi = kc * 128, min((kc + 1) * 128, len(ki))
                nrow = max(0, hi - lo)
                if nrow:
                    vap[:nrow, coff * VA : coff * VA + H] = value[gi, ki[lo:hi], :]
                    vap[:nrow, coff * VA + H] = 1.0
                    kmp[:nrow, coff] = 1.0
                coff += 1
        in_maps.append(
            {
                "qT": qTp,
                "kT": kTp,
                "vaug": vap,
                "w1": W1,
                "b1T": b1T,
                "zmr": zmr,
                "kmT": kmp,
                "b2c": b2c,
            }
        )
    return in_maps, assign, q_idx, QN, KK


def kernel(query, key, value, q_mask, k_mask, W1, b1, W2, b2):
    in_maps, assign, q_idx, QN, KK = _prep(
        query, key, value, q_mask, k_mask, W1, b1, W2, b2
    )
    nc = _build(QN, KK)
    res = run_bass_kernel_spmd(nc, in_maps, core_ids=list(range(N_CORES)))
    out = np.zeros((B, S1, H), np.float32)
    for c in range(N_CORES):
        yv = res.results[c]["y"]
        for s in range(BPC):
            gi = assign[c, s]
            qi = q_idx[gi]
            if len(qi):
                out[gi, qi, :] = yv[s, : len(qi), :]
    return out


def traced_single_core(query, key, value, q_mask, k_mask, W1, b1, W2, b2, core=0):
    """Run one core's share with NTFF tracing; returns (out, exec_time_ns)."""
    in_maps, assign, q_idx, QN, KK = _prep(
        query, key, value, q_mask, k_mask, W1, b1, W2, b2
    )
    nc = _build(QN, KK)
    res = run_bass_kernel_spmd(nc, [in_maps[core]], core_ids=[0], trace=True)
    out = np.zeros((B, S1, H), np.float32)
    yv = res.results[0]["y"]
    for s in range(BPC):
        gi = assign[core, s]
        qi = q_idx[gi]
        if len(qi):
            out[gi, qi, :] = yv[s, : len(qi), :]
    return out, res.exec_time_ns


# revision 10
# speedup vs baseline: 1.4264x; 1.0015x over previous
"""Trainium2 Bass kernel for nn_AttentionLayer (additive/Bahdanau-style attention).

Reference computation:
  qp = query @ W1[:H] ; kp = key @ W1[H:]          # [B,S1,HM], [B,S2,HM]
  hid = relu(qp[:,:,None,:] + kp[:,None,:,:] + b1)  # [B,S1,S2,HM]
  scores = hid @ W2 + b2                            # [B,S1,S2]
  A = where(qmask*kmask==0, 0, exp(scores))
  out = (A / clip(A.sum(-1), 2e-15)) @ value        # [B,S1,H]

Sharding: data-parallel over batch, 2 batches per core on 8 cores.

Sparsity: masked q rows produce all-zero output rows and masked k columns
contribute exactly zero to every sum, so the host compacts both dimensions
per batch (keeping only mask==1 rows, zero-padded to per-SLOT capacities)
and scatters the result back. Batches are sorted by kept-q count and paired
big-with-small across the two per-core slots, so slot capacities hug the
actual counts. The device still applies the k-mask (padding lanes carry
mask 0), so results are exact up to bf16 rounding of W1/query/key/hid.

Device mapping (per core, 2 batch slots):
  stage0 (PE):   qpT[h,q], kpT[h,k] projections (+b1 via ACT Identity evict)
  pair  (DVE/ACT): hidT[h,k] = relu(kpT_hb + qpT_hb[:,q]) per (slot,q,hb) as
                 one fused per-partition-scalar op; both slots write one wide
                 [128, K0+K1] tile consumed by a single PE matmul
  scores (PE):   matmul with sliding-window weights Z(hb,j) = zmr[:,31-j:63-j]
                 (W2 h-block parked at column 31) -> scores row lands on PSUM
                 partition q=32*cg+j via tile_position col groups, fp32 accum
  post (per slot): Exp(+b2); PE transpose -> A_T[k,q]; k_mask on PSUM evict;
                 out_un = A_T.T @ [value | ones] (ones column = row-sum);
                 out = out_un * 1/clip(rowsum, 2e-15) per-partition.
"""

import os
import sys

import numpy as np

for _p in ("/opt/trn_rl_repo",):
    if os.path.isdir(_p) and _p not in sys.path:
        sys.path.insert(0, _p)

import ml_dtypes  # noqa: E402
import concourse.bass as bass  # noqa: E402
import concourse.mybir as mybir  # noqa: E402
import concourse.tile as tile  # noqa: E402
from concourse import bacc  # noqa: E402
from concourse.bass_utils import run_bass_kernel_spmd  # noqa: E402
from concourse.masks import make_identity  # noqa: E402

B, S1, S2, H, HM = 16, 128, 256, 256, 512
N_CORES = 8
BPC = B // N_CORES  # batch slots per core
NHB = HM // 128  # h blocks
NDC = H // 128  # d chunks (projection contraction)
VA = H + 2  # value dims + ones column + pad
ACT_PAT = (0, 3, 6, 9, 13, 16, 19, 22, 26, 29)  # cnt%32 residues on ACT

FP32 = mybir.dt.float32
BF16 = mybir.dt.bfloat16
ADD = mybir.AluOpType.add
MAX = mybir.AluOpType.max
RELU = mybir.ActivationFunctionType.Relu
EXP = mybir.ActivationFunctionType.Exp
IDENT_F = mybir.ActivationFunctionType.Identity

_cache: dict = {}


def _build(QN, KK):
    """Build + compile the per-core program.

    QN/KK: per-slot q and k capacities, e.g. QN=(80, 64), KK=(144, 136).
    """
    ck = (QN, KK)
    if ck in _cache:
        return _cache[ck]

    CGC = [(q + 31) // 32 for q in QN]  # col groups per slot
    QP32 = [c * 32 for c in CGC]
    KC = [(k + 127) // 128 for k in KK]  # k chunks per slot
    KOFF = [0, KK[0]]  # slot column offsets in wide tiles
    KW = KK[0] + KK[1]  # wide k extent
    KCT = sum(KC)
    QTW = NDC * (QN[0] + QN[1])  # packed qT width
    KTW = NDC * (KK[0] + KK[1])  # packed kT width

    nc = bacc.Bacc("TRN2", target_bir_lowering=False, debug=False, num_devices=1)

    qT = nc.dram_tensor("qT", [128, QTW], BF16, kind="ExternalInput")
    kT = nc.dram_tensor("kT", [128, KTW], BF16, kind="ExternalInput")
    vaug = nc.dram_tensor("vaug", [128, KCT * VA], FP32, kind="ExternalInput")
    w1 = nc.dram_tensor("w1", [2 * H, HM], BF16, kind="ExternalInput")
    misc = nc.dram_tensor("misc", [128, NHB + KCT + 1], FP32, kind="ExternalInput")
    zmr = nc.dram_tensor("zmr", [128, NHB * 64], BF16, kind="ExternalInput")
    y = nc.dram_tensor("y", [BPC, QP32[0], H], FP32, kind="ExternalOutput")

    with tile.TileContext(nc) as tc:
        with (
            tc.tile_pool(name="const", bufs=1) as cp,
            tc.tile_pool(name="acts", bufs=1) as ap,
            tc.tile_pool(name="hid", bufs=28) as hp,
            tc.tile_pool(name="small", bufs=2) as sp,
            tc.tile_pool(name="psA", bufs=2, space=bass.MemorySpace.PSUM) as psA,
            tc.tile_pool(name="psS", bufs=1, space=bass.MemorySpace.PSUM) as psS,
            tc.tile_pool(name="psB", bufs=2, space=bass.MemorySpace.PSUM) as psB,
        ):
            # ---------------- inputs to SBUF ----------------
            # Warmup: trigger the ACT table load immediately (no DMA deps) so
            # the ~1.3us PSEUDO_LOAD_ACT_FUNC_SET overlaps the input DMAs.
            warm = cp.tile([1, 2], FP32, name="warm", tag="warm")
            nc.vector.memset(warm[:], 0.0)
            nc.scalar.activation(warm[:], warm[:], RELU)
            # Few wide DMAs (HWDGE issue is ~0.6us each on the sync
            # sequencer); critical path (w1k/kT -> first projections) first,
            # big loads split in halves so two queues stream in parallel.
            w1t = {}
            for dc in range(NDC):
                tk = cp.tile([128, HM], BF16, name=f"w1ks{dc}", tag=f"w1ks{dc}")
                nc.sync.dma_start(tk[:], w1.ap()[H + dc * 128 : H + (dc + 1) * 128, :])
                w1t["k", dc] = tk
            kT_all = cp.tile([128, KTW], BF16, name="kT_all", tag="kT_all")
            hw = KTW // 2
            nc.sync.dma_start(kT_all[:, :hw], kT.ap()[:, :hw])
            nc.sync.dma_start(kT_all[:, hw:], kT.ap()[:, hw:])
            for dc in range(NDC):
                tq = cp.tile([128, HM], BF16, name=f"w1qs{dc}", tag=f"w1qs{dc}")
                nc.sync.dma_start(tq[:], w1.ap()[dc * 128 : (dc + 1) * 128, :])
                w1t["q", dc] = tq
            qT_all = cp.tile([128, QTW], BF16, name="qT_all", tag="qT_all")
            nc.sync.dma_start(qT_all[:], qT.ap())
            misc_all = cp.tile([128, NHB + KCT + 1], FP32, name="misc_all", tag="misc_all")
            nc.sync.dma_start(misc_all[:], misc.ap())
            zm = cp.tile([128, NHB * 64], BF16, name="zm", tag="zm")
            nc.sync.dma_start(zm[:], zmr.ap())
            va_all = cp.tile([128, KCT * VA], FP32, name="va_all", tag="va_all")
            vw = (KCT * VA) // 2
            nc.gpsimd.dma_start(va_all[:, :vw], vaug.ap()[:, :vw])
            nc.gpsimd.dma_start(va_all[:, vw:], vaug.ap()[:, vw:])
            ident = cp.tile([128, 128], FP32, name="ident", tag="ident")
            make_identity(nc, ident[:])

            w1q, w1k, qT_sb, kT_sb, va_sb, km_sb, b1_sb = {}, {}, {}, {}, {}, {}, {}
            qoff = koff = 0
            for b in range(BPC):
                for dc in range(NDC):
                    kT_sb[b, dc] = kT_all[:, koff : koff + KK[b]]
                    qT_sb[b, dc] = qT_all[:, qoff : qoff + QN[b]]
                    qoff += QN[b]
                    koff += KK[b]
            coff = 0
            for b in range(BPC):
                for kc in range(KC[b]):
                    va_sb[b, kc] = va_all[:, coff * VA : (coff + 1) * VA]
                    km_sb[b, kc] = misc_all[:, NHB + coff : NHB + coff + 1]
                    coff += 1
            for hb in range(NHB):
                b1_sb[hb] = misc_all[:, hb : hb + 1]
            b2_sb = misc_all[:, NHB + KCT : NHB + KCT + 1]
            for dc in range(NDC):
                for hb in range(NHB):
                    w1q[dc, hb] = w1t["q", dc][:, hb * 128 : (hb + 1) * 128]
                    w1k[dc, hb] = w1t["k", dc][:, hb * 128 : (hb + 1) * 128]

            # ---------------- stage 0: projections ----------------
            qpT, kpB = {}, {}
            for hb in range(NHB):
                for b in range(BPC):
                    ps = psA.tile([128, max(KK)], FP32, name="proj", tag="proj")
                    for dc in range(NDC):
                        nc.tensor.matmul(
                            ps[:, : QN[b]],
                            w1q[dc, hb],
                            qT_sb[b, dc],
                            start=(dc == 0),
                            stop=(dc == NDC - 1),
                        )
                    t = ap.tile([128, QN[b]], FP32, name=f"qpT{b}{hb}", tag=f"qpT{b}{hb}")
                    nc.scalar.activation(t[:], ps[:, : QN[b]], IDENT_F, bias=b1_sb[hb])
                    qpT[b, hb] = t
                    ps2 = psA.tile([128, max(KK)], FP32, name="proj", tag="proj")
                    for dc in range(NDC):
                        nc.tensor.matmul(
                            ps2[:, : KK[b]],
                            w1k[dc, hb],
                            kT_sb[b, dc],
                            start=(dc == 0),
                            stop=(dc == NDC - 1),
                        )
                    t2 = ap.tile([128, KK[b]], BF16, name=f"kpB{b}{hb}", tag=f"kpB{b}{hb}")
                    nc.scalar.activation(t2[:], ps2[:, : KK[b]], IDENT_F, bias=b1_sb[hb])
                    kpB[b, hb] = t2

            # ---------------- pair stage + score reduce ----------------
            scores = psS.tile([128, KW], FP32, name="scps", tag="scps")
            cnt = 0
            for hb in range(NHB):
                for j in range(32):
                    w_ap = zm[:, hb * 64 + (31 - j) : hb * 64 + (63 - j)]
                    for cg in range(CGC[0]):
                        q = cg * 32 + j
                        if q >= QN[0]:
                            continue
                        hid = hp.tile([128, KW], BF16, name="hid", tag="hid")
                        nw = KK[0]
                        for b in range(BPC):
                            if q >= QN[b]:
                                continue
                            nw = KOFF[b] + KK[b]
                            qcol = qpT[b, hb][:, q : q + 1]
                            dst = hid[:, KOFF[b] : KOFF[b] + KK[b]]
                            if cnt % 32 in ACT_PAT:
                                nc.scalar.activation(
                                    dst, kpB[b, hb][:], RELU, bias=qcol, scale=1.0
                                )
                            else:
                                nc.vector.tensor_scalar(
                                    dst, kpB[b, hb][:], qcol, 0.0, ADD, MAX
                                )
                            cnt += 1
                        jlast = min(31, QN[0] - 1 - cg * 32)
                        nc.tensor.matmul(
                            scores[cg * 32 : (cg + 1) * 32, :nw],
                            w_ap,
                            hid[:, :nw],
                            start=(hb == 0 and j == 0),
                            stop=(hb == NHB - 1 and j == jlast),
                            tile_position=(0, cg * 32),
                        )

            # ---------------- per-slot: exp / transpose / mask / value ----------------
            # Emitted phase-interleaved across the two slots so the serial
            # chain of one slot overlaps the other's on different engines.
            A, AT, pso = {}, {}, {}
            for b in range(BPC):
                A[b] = ap.tile([128, KK[b]], FP32, name=f"Aexp{b}", tag=f"Aexp{b}")
                nc.scalar.activation(
                    A[b][: QP32[b], :],
                    scores[: QP32[b], KOFF[b] : KOFF[b] + KK[b]],
                    EXP,
                    bias=b2_sb[0 : QP32[b], :],
                    scale=1.0,
                )
            for b in range(BPC):
                for kc in range(KC[b]):
                    kw = min(128, KK[b] - kc * 128)
                    pst = psB.tile([128, QP32[b]], FP32, name="trps", tag="trps")
                    nc.tensor.transpose(
                        pst[:kw, :],
                        A[b][: QP32[b], kc * 128 : kc * 128 + kw],
                        ident[: QP32[b], : QP32[b]],
                    )
                    at = ap.tile([128, QP32[b]], FP32, name=f"AT{b}{kc}", tag=f"AT{b}{kc}")
                    nc.scalar.activation(
                        at[:kw, :], pst[:kw, :], IDENT_F, scale=km_sb[b, kc][0:kw, :]
                    )
                    AT[b, kc] = at
            for b in range(BPC):
                pso[b] = psB.tile([128, VA], FP32, name=f"oun{b}", tag=f"oun{b}", bufs=1)
                for kc in range(KC[b]):
                    kw = min(128, KK[b] - kc * 128)
                    nc.tensor.matmul(
                        pso[b][: QP32[b], :],
                        AT[b, kc][:kw, :],
                        va_sb[b, kc][0:kw, :],
                        start=(kc == 0),
                        stop=(kc == KC[b] - 1),
                    )
            for b in range(BPC):
                qp32 = QP32[b]
                asum = sp.tile([128, 1], FP32, name="asum", tag="asum")
                nc.vector.tensor_scalar_max(asum[:qp32, :], pso[b][:qp32, H : H + 1], 2e-15)
                rec = sp.tile([128, 1], FP32, name="rec", tag="rec")
                nc.vector.reciprocal(rec[:qp32, :], asum[:qp32, :])
                outt = ap.tile([128, H], FP32, name=f"out{b}", tag=f"out{b}")
                nc.scalar.activation(
                    outt[:qp32, :], pso[b][:qp32, 0:H], IDENT_F, scale=rec[:qp32, 0:1]
                )
                nc.sync.dma_start(y.ap()[b, 0:qp32, :], outt[:qp32, :])

    nc.compile()
    _cache[ck] = nc
    return nc


def _r(x, m):
    return ((max(int(x), 1) + m - 1) // m) * m


def _prep(query, key, value, q_mask, k_mask, W1, b1, W2, b2):
    query = np.asarray(query, np.float32)
    key = np.asarray(key, np.float32)
    value = np.asarray(value, np.float32)
    q_mask = np.asarray(q_mask, np.float32)
    k_mask = np.asarray(k_mask, np.float32)
    W1 = np.ascontiguousarray(np.asarray(W1, ml_dtypes.bfloat16))
    b1 = np.asarray(b1, np.float32)
    W2 = np.asarray(W2, np.float32)
    b2 = np.asarray(b2, np.float32)

    q_idx = [np.nonzero(q_mask[i] != 0)[0] for i in range(B)]
    k_idx = [np.nonzero(k_mask[i] != 0)[0] for i in range(B)]
    qn = np.array([len(ix) for ix in q_idx])

    # Slot assignment: sort by kept-q count; 8 largest -> slot 0, rest -> slot 1.
    order = np.argsort(-qn, kind="stable")
    slot_batches = [list(order[:N_CORES]), list(order[N_CORES:])]
    QN = tuple(_r(max(len(q_idx[i]) for i in slot_batches[s]), 2) for s in range(BPC))
    KK = tuple(_r(max(len(k_idx[i]) for i in slot_batches[s]), 2) for s in range(BPC))
    KC = [(k + 127) // 128 for k in KK]
    KCT = sum(KC)

    zmr = np.zeros((128, NHB * 64), np.float32)
    for hb in range(NHB):
        zmr[:, hb * 64 + 31] = W2[hb * 128 : (hb + 1) * 128, 0]
    zmr = zmr.astype(ml_dtypes.bfloat16)

    assign = {}  # (core, slot) -> global batch idx
    in_maps = []
    QTW = NDC * (QN[0] + QN[1])
    KTW = NDC * (KK[0] + KK[1])
    for c in range(N_CORES):
        qTp = np.zeros((128, QTW), ml_dtypes.bfloat16)
        kTp = np.zeros((128, KTW), ml_dtypes.bfloat16)
        vap = np.zeros((128, KCT * VA), np.float32)
        miscp = np.zeros((128, NHB + KCT + 1), np.float32)
        miscp[:, :NHB] = b1.reshape(NHB, 128).T
        miscp[:, NHB + KCT] = float(b2[0])
        qoff = koff = coff = 0
        for s in range(BPC):
            gi = slot_batches[s][c]
            assign[c, s] = gi
            qi, ki = q_idx[gi], k_idx[gi]
            for dc in range(NDC):
                if len(qi):
                    qTp[:, qoff : qoff + len(qi)] = query[
                        gi, qi, dc * 128 : (dc + 1) * 128
                    ].T.astype(ml_dtypes.bfloat16)
                if len(ki):
                    kTp[:, koff : koff + len(ki)] = key[
                        gi, ki, dc * 128 : (dc + 1) * 128
                    ].T.astype(ml_dtypes.bfloat16)
                qoff += QN[s]
                koff += KK[s]
            for kc in range(KC[s]):
                lo, hi = kc * 128, min((kc + 1) * 128, len(ki))
                nrow = max(0, hi - lo)
                if nrow:
                    vap[:nrow, coff * VA : coff * VA + H] = value[gi, ki[lo:hi], :]
                    vap[:nrow, coff * VA + H] = 1.0
                    miscp[:nrow, NHB + coff] = 1.0
                coff += 1
        in_maps.append(
            {
                "qT": qTp,
                "kT": kTp,
                "vaug": vap,
                "w1": W1,
                "zmr": zmr,
                "misc": miscp,
            }
        )
    return in_maps, assign, q_idx, QN, KK


def kernel(query, key, value, q_mask, k_mask, W1, b1, W2, b2):
    in_maps, assign, q_idx, QN, KK = _prep(
        query, key, value, q_mask, k_mask, W1, b1, W2, b2
    )
    nc = _build(QN, KK)
    res = run_bass_kernel_spmd(nc, in_maps, core_ids=list(range(N_CORES)))
    out = np.zeros((B, S1, H), np.float32)
    for c in range(N_CORES):
        yv = res.results[c]["y"]
        for s in range(BPC):
            gi = assign[c, s]
            qi = q_idx[gi]
            if len(qi):
                out[gi, qi, :] = yv[s, : len(qi), :]
    return out


def traced_single_core(query, key, value, q_mask, k_mask, W1, b1, W2, b2, core=0):
    """Run one core's share with NTFF tracing; returns (out, exec_time_ns)."""
    in_maps, assign, q_idx, QN, KK = _prep(
        query, key, value, q_mask, k_mask, W1, b1, W2, b2
    )
    nc = _build(QN, KK)
    res = run_bass_kernel_spmd(nc, [in_maps[core]], core_ids=[0], trace=True)
    out = np.zeros((B, S1, H), np.float32)
    yv = res.results[0]["y"]
    for s in range(BPC):
        gi = assign[core, s]
        qi = q_idx[gi]
        if len(qi):
            out[gi, qi, :] = yv[s, : len(qi), :]
    return out, res.exec_time_ns
